# revision 1
# baseline (speedup 1.0000x reference)
import math
import sys

sys.path.insert(0, "/opt/trn_rl_repo")

import numpy as np
import ml_dtypes

bf16np = ml_dtypes.bfloat16

# ---------------- problem constants (hardcoded; kernel.py must be self-contained) ----
B, T, S, D, H, L, DFF, IN, PERIOD = 16, 600, 600, 1024, 8, 8, 4096, 52, 25
HD = D // H          # 128
NC = 8               # cores
BC = B // NC         # 2 batches per core
DI = D // 128        # 8 i-tiles
DFI = DFF // 128     # 32
INV = 1.0 / math.sqrt(HD)
TK = 1 + T           # 601 keys (adapter + T)
# chunks of the 600-wide token dim (psum bank = 512 f32)
TCH = [(0, 512), (512, 88)]
KCH = [(0, 512), (512, 89)]          # 601-wide
KT = [(0, 128), (128, 128), (256, 128), (384, 128), (512, 89)]  # key tiles of 601

_cache = {}


def _build():
    """Build the per-core Bass graph (SPMD; same program all 8 cores)."""
    from concourse import bacc, mybir
    import concourse.bass as bass
    import concourse.bass_isa as bass_isa
    import concourse.tile as tile

    f32 = mybir.dt.float32
    bf = mybir.dt.bfloat16
    i32 = mybir.dt.int32
    AF = mybir.ActivationFunctionType
    OP = mybir.AluOpType
    RED = bass_isa.ReduceOp

    nc = bacc.Bacc("TRN2", target_bir_lowering=False, debug=False, num_devices=NC)

    def din(name, shape, dt=f32):
        return nc.dram_tensor(name, shape, dt, kind="ExternalInput").ap()

    # ---- DRAM inputs (host-prepped layouts) ----
    xT = din("xT", [BC, IN, T], bf)              # x transposed, bf16
    memT = din("memT", [BC, D, T], bf)           # memory transposed, bf16
    tsf = din("tsf", [1, BC])                    # timesteps as f32
    efm = din("efm", [128, DI])                  # e/(2pi) tiled per i-tile col
    phs = din("phs", [128, DI])                  # phase (0 / .25)
    peT = din("peT", [D, T])                     # pe.T + b_in  (f32)
    w_inT = din("w_inT", [IN, D], bf)
    te_w1T = din("te_w1T", [D, D], bf)
    te_w2T = din("te_w2T", [D, D], bf)
    te_b1t = din("te_b1t", [128, DI])
    te_b2t = din("te_b2t", [128, DI])
    sa_wqkvT = din("sa_wqkvT", [L, D, 3 * D], bf)
    sa_bqkvt = din("sa_bqkvt", [L, 128, 3 * DI])  # pre-tiled [128, 24]; q-part prescaled by INV
    sa_bvrow = din("sa_bvrow", [L, 1, D], bf)     # v-bias as row (for ones-MM trick)
    sa_woT = din("sa_woT", [L, D, D], bf)
    sa_bot = din("sa_bot", [L, 128, DI])
    ca_wqkvT = din("ca_wqkvT", [L, D, 3 * D], bf)
    ca_bqkvt = din("ca_bqkvt", [L, 128, 3 * DI])
    ca_woT = din("ca_woT", [L, D, D], bf)
    ca_bot = din("ca_bot", [L, 128, DI])
    ff_w1T = din("ff_w1T", [L, D, DFF], bf)
    ff_b1t = din("ff_b1t", [L, 128, DFI])
    ff_w2T = din("ff_w2T", [L, DFF, D], bf)
    ff_b2t = din("ff_b2t", [L, 128, DI])
    lngt = din("lngt", [L, 3, 128, DI])
    lnbt = din("lnbt", [L, 3, 128, DI])
    biasT = din("biasT", [H, TK, T])             # SA alibi bias transposed [h, k, q], f32
    w_outT = din("w_outT", [D, IN], bf)
    b_out = din("b_out", [IN, 1])
    out_d = nc.dram_tensor("out", [BC, IN, T], f32, kind="ExternalOutput").ap()

    with tile.TileContext(nc) as tc:
        res = tc.alloc_tile_pool(name="res", bufs=1)      # persistent
        w6 = tc.alloc_tile_pool(name="w6", bufs=32)       # bf16 [128,601] q/k/pt/attn ws
        fa = tc.alloc_tile_pool(name="fa", bufs=17)       # bf16 [128,601] ffa tiles
        hqp = tc.alloc_tile_pool(name="hqp", bufs=12)     # bf16 [128,601] LN targets
        mxp = tc.alloc_tile_pool(name="mxp", bufs=9)      # bf16 [128,601] mem staging
        vs = tc.alloc_tile_pool(name="vs", bufs=6)        # bf16 [128,1024] V tiles
        f6 = tc.alloc_tile_pool(name="f6", bufs=12)       # f32 [128,600] workspace
        wg = tc.alloc_tile_pool(name="wg", bufs=22)       # bf16 [128,512] weights
        bp = tc.alloc_tile_pool(name="bp", bufs=2)        # f32 [128,600] bias/pe stream
        sm = tc.alloc_tile_pool(name="sm", bufs=1)        # small persistents
        pk = tc.alloc_tile_pool(name="pk", bufs=6, space="PSUM")

        # persistent residual f32 and bf16 carrier (adapter col 0) - ONE batch at a time
        hf1 = [res.tile([128, T], f32, tag=f"hf_{i}", name=f"hf_{i}") for i in range(DI)]
        hx1 = [res.tile([128, TK], bf, tag=f"hx_{i}", name=f"hx_{i}") for i in range(DI)]
        hf = [hf1 for _ in range(BC)]
        hx = [hx1 for _ in range(BC)]

        ones_f = sm.tile([1, 128], f32, tag="ones_f", name="ones_f")
        nc.vector.memset(ones_f[:], 1.0)
        ones_b = sm.tile([1, 128], bf, tag="ones_b", name="ones_b")
        nc.vector.memset(ones_b[:], 1.0)
        eft = sm.tile([128, DI], f32, tag="eft", name="eft")
        nc.sync.dma_start(eft[:], efm[:])
        pht = sm.tile([128, DI], f32, tag="pht", name="pht")
        nc.sync.dma_start(pht[:], phs[:])
        tst = sm.tile([1, BC], f32, tag="tst", name="tst")
        eps_t = sm.tile([128, 1], f32, tag="eps_t", name="eps_t")
        nc.vector.memset(eps_t[:], 1e-5)
        nc.sync.dma_start(tst[:], tsf[:])
        adp = [sm.tile([128, BC], bf, tag=f"adp{i}", name=f"adp{i}") for i in range(DI)]  # adapter bf16

        # ---------- timestep embedding ----------
        ptb = pk.tile([128, BC], f32, tag="pk", name="ptb")
        nc.tensor.matmul(ptb[:], ones_f[:], tst[:], start=True, stop=True)  # t bcast f32
        temb = []
        for i in range(DI):
            y = sm.tile([128, BC], f32, tag=f"y{i}", name=f"y{i}")
            nc.vector.tensor_scalar_mul(y[:], ptb[:], eft[:, i : i + 1])
            nc.vector.tensor_scalar_add(y[:], y[:], pht[:, i : i + 1])
            yi = sm.tile([128, BC], i32, tag=f"yi{i}", name=f"yi{i}")
            nc.vector.tensor_copy(yi[:], y[:])
            yr = sm.tile([128, BC], f32, tag=f"yr{i}", name=f"yr{i}")
            nc.vector.tensor_copy(yr[:], yi[:])
            fr = sm.tile([128, BC], f32, tag=f"fr{i}", name=f"fr{i}")
            nc.vector.tensor_sub(fr[:], y[:], yr[:])
            tb = sm.tile([128, BC], bf, tag=f"tb{i}", name=f"tb{i}")
            nc.scalar.activation(tb[:], fr[:], AF.Sin, scale=2 * math.pi)
            temb.append(tb)

        def mlp1024(wT_d, bt_d, ins, act, outs_tag):
            """[D,D] proj on BC-wide f-major input tiles. Returns 8 bf16 [128,BC] tiles."""
            bt = sm.tile([128, DI], f32, tag=outs_tag + "_b", name=outs_tag + "_b")
            nc.sync.dma_start(bt[:], bt_d[:])
            outs = []
            for og in range(2):
                wts = []
                for i in range(DI):
                    w = wg.tile([128, 512], bf, tag="wg", name="wg")
                    nc.sync.dma_start(w[:], wT_d[i * 128 : (i + 1) * 128, og * 512 : (og + 1) * 512])
                    wts.append(w)
                for ot in range(4):
                    o = og * 4 + ot
                    p = pk.tile([128, BC], f32, tag="pk", name="pmlp")
                    for i in range(DI):
                        nc.tensor.matmul(p[:], wts[i][:, ot * 128 : (ot + 1) * 128], ins[i][:],
                                         start=(i == 0), stop=(i == DI - 1))
                    ob = sm.tile([128, BC], bf, tag=f"{outs_tag}{o}", name=f"{outs_tag}{o}")
                    nc.scalar.activation(ob[:], p[:], act, bias=bt[:, o : o + 1])
                    outs.append(ob)
            return outs

        z1 = mlp1024(te_w1T, te_b1t, temb, AF.Silu, "z1")
        z2 = mlp1024(te_w2T, te_b2t, z1, AF.Identity, "z2")
        for i in range(DI):
            nc.vector.tensor_copy(adp[i][:], z2[i][:])

        # ---------- helpers ----------
        def layernorm(b, g_ap, b_ap, tgt, tgt_off):
            """LN over features of X held in hf[b] (in place); bf16 copy to tgt[o][:, off:off+T]."""
            sacc = f6.tile([128, T], f32, tag="f6", name="f6")
            nc.vector.tensor_tensor(sacc[:], hf[b][0][:], hf[b][1][:], OP.add)
            for o in range(2, DI):
                nc.vector.tensor_tensor(sacc[:], sacc[:], hf[b][o][:], OP.add)
            qacc = f6.tile([128, T], f32, tag="f6", name="f6")
            tmp = f6.tile([128, T], f32, tag="f6", name="f6")
            nc.vector.tensor_tensor(qacc[:], hf[b][0][:], hf[b][0][:], OP.mult)
            for o in range(1, DI):
                nc.vector.tensor_tensor(tmp[:], hf[b][o][:], hf[b][o][:], OP.mult)
                nc.vector.tensor_tensor(qacc[:], qacc[:], tmp[:], OP.add)
            s1 = f6.tile([128, T], f32, tag="f6", name="f6")
            nc.gpsimd.partition_all_reduce(s1[:], sacc[:], channels=128, reduce_op=RED.add)
            s2 = f6.tile([128, T], f32, tag="f6", name="f6")
            nc.gpsimd.partition_all_reduce(s2[:], qacc[:], channels=128, reduce_op=RED.add)
            m = f6.tile([128, T], f32, tag="f6", name="f6")
            nc.vector.tensor_scalar_mul(m[:], s1[:], 1.0 / D)
            m2 = f6.tile([128, T], f32, tag="f6", name="f6")
            nc.vector.tensor_tensor(m2[:], m[:], m[:], OP.mult)
            var = f6.tile([128, T], f32, tag="f6", name="f6")
            nc.vector.scalar_tensor_tensor(var[:], s2[:], 1.0 / D, m2[:], OP.mult, OP.subtract)
            sd = f6.tile([128, T], f32, tag="f6", name="f6")
            nc.scalar.activation(sd[:], var[:], AF.Sqrt, bias=eps_t[:])
            r = f6.tile([128, T], f32, tag="f6", name="f6")
            nc.vector.reciprocal(r[:], sd[:])
            mr = f6.tile([128, T], f32, tag="f6", name="f6")
            nc.vector.tensor_tensor(mr[:], m[:], r[:], OP.mult)
            for o in range(DI):
                t1 = f6.tile([128, T], f32, tag="f6", name="f6")
                nc.vector.tensor_tensor(t1[:], hf[b][o][:], r[:], OP.mult)
                nc.vector.tensor_tensor(t1[:], t1[:], mr[:], OP.subtract)
                nc.scalar.activation(hf[b][o][:], t1[:], AF.Identity,
                                     bias=b_ap[:, o : o + 1], scale=g_ap[:, o : o + 1])
                nc.vector.tensor_copy(tgt[o][:, tgt_off : tgt_off + T], hf[b][o][:])

        def proj_res(b, wT_l, bot_ap, rhs_tiles, rhs_off):
            """out-proj [D,D] + bias + residual into hf[b] (X pre-LN)."""
            for og in range(2):
                wts = []
                for i in range(DI):
                    w = wg.tile([128, 512], bf, tag="wg", name="wg")
                    nc.sync.dma_start(w[:], wT_l[i * 128 : (i + 1) * 128, og * 512 : (og + 1) * 512])
                    wts.append(w)
                for ot in range(4):
                    o = og * 4 + ot
                    for c0, cw in TCH:
                        p = pk.tile([128, 512], f32, tag="pk", name="pk")
                        for i in range(DI):
                            nc.tensor.matmul(p[:, :cw], wts[i][:, ot * 128 : (ot + 1) * 128],
                                             rhs_tiles[i][:, rhs_off + c0 : rhs_off + c0 + cw],
                                             start=(i == 0), stop=(i == DI - 1))
                        nc.vector.scalar_tensor_tensor(hf[b][o][:, c0 : c0 + cw], p[:, :cw],
                                                       bot_ap[:, o : o + 1], hf[b][o][:, c0 : c0 + cw],
                                                       OP.add, OP.add)

        # per-layer bias tiles (re-DMAed each (b, l))
        sa_bq = sm.tile([128, 3 * DI], f32, tag="sa_bq", name="sa_bq")
        ca_bq = sm.tile([128, 3 * DI], f32, tag="ca_bq", name="ca_bq")
        sa_bo_t = sm.tile([128, DI], f32, tag="sa_bo_t", name="sa_bo_t")
        ca_bo_t = sm.tile([128, DI], f32, tag="ca_bo_t", name="ca_bo_t")
        f_b1 = sm.tile([128, DFI], f32, tag="f_b1", name="f_b1")
        f_b2 = sm.tile([128, DI], f32, tag="f_b2", name="f_b2")
        lng = [sm.tile([128, DI], f32, tag=f"lng{k}", name=f"lng{k}") for k in range(3)]
        lnb = [sm.tile([128, DI], f32, tag=f"lnb{k}", name=f"lnb{k}") for k in range(3)]
        bvr = sm.tile([1, D], bf, tag="bvr", name="bvr")
        bo_t = sm.tile([IN, 1], f32, tag="bo_t", name="bo_t")
        nc.sync.dma_start(bo_t[:], b_out[:])

        # ================= batch-serial main =================
        for b in range(BC):
            # ---------- input projection + pe ----------
            xb = sm.tile([IN, T], bf, tag="xb", name="xb")
            nc.sync.dma_start(xb[:], xT[b])
            for og in range(2):
                wts = []
                for ot in range(4):
                    o = og * 4 + ot
                    w = wg.tile([IN, 128], bf, tag="wgin", name="wgin")
                    nc.sync.dma_start(w[:], w_inT[:, o * 128 : (o + 1) * 128])
                    wts.append(w)
                for ot in range(4):
                    o = og * 4 + ot
                    pe_t = bp.tile([128, T], f32, tag="bp", name="bp")
                    nc.sync.dma_start(pe_t[:], peT[o * 128 : (o + 1) * 128, :])
                    for c0, cw in TCH:
                        p = pk.tile([128, 512], f32, tag="pk", name="pk")
                        nc.tensor.matmul(p[:, :cw], wts[ot][:], xb[:, c0 : c0 + cw],
                                         start=True, stop=True)
                        nc.vector.tensor_tensor(hf[b][o][:, c0 : c0 + cw], p[:, :cw],
                                                pe_t[:, c0 : c0 + cw], OP.add)
                    nc.vector.tensor_copy(hx[b][o][:, 1:], hf[b][o][:])
                    nc.vector.tensor_copy(hx[b][o][:, 0:1], adp[o][:, b : b + 1])

            for l in range(L):
                nc.sync.dma_start(sa_bq[:], sa_bqkvt[l])
                nc.sync.dma_start(ca_bq[:], ca_bqkvt[l])
                nc.sync.dma_start(sa_bo_t[:], sa_bot[l])
                nc.sync.dma_start(ca_bo_t[:], ca_bot[l])
                nc.sync.dma_start(f_b1[:], ff_b1t[l])
                nc.sync.dma_start(f_b2[:], ff_b2t[l])
                for k in range(3):
                    nc.sync.dma_start(lng[k][:], lngt[l, k])
                    nc.sync.dma_start(lnb[k][:], lnbt[l, k])
                nc.sync.dma_start(bvr[:], sa_bvrow[l])

                # ================= self-attention =================
                qsb = []
                ksb = []
                for og in range(4):
                    wts = []
                    for i in range(DI):
                        w = wg.tile([128, 512], bf, tag="wg", name="wg")
                        nc.sync.dma_start(w[:], sa_wqkvT[l, i * 128 : (i + 1) * 128,
                                                         og * 512 : (og + 1) * 512])
                        wts.append(w)
                    is_q = og < 2
                    for ot in range(4):
                        o = og * 4 + ot
                        dst = w6.tile([128, TK], bf, tag="w6", name="w6")
                        chunks = TCH if is_q else KCH
                        r0 = 1 if is_q else 0
                        for c0, cw in chunks:
                            p = pk.tile([128, 512], f32, tag="pk", name="pk")
                            for i in range(DI):
                                nc.tensor.matmul(p[:, :cw], wts[i][:, ot * 128 : (ot + 1) * 128],
                                                 hx[b][i][:, r0 + c0 : r0 + c0 + cw],
                                                 start=(i == 0), stop=(i == DI - 1))
                            nc.scalar.activation(dst[:, c0 : c0 + cw], p[:, :cw], AF.Identity,
                                                 bias=sa_bq[:, o : o + 1],
                                                 scale=INV if is_q else 1.0)
                        (qsb if is_q else ksb).append(dst)
                vsb = [vs.tile([128, 1024], bf, tag="vs", name="vs") for _ in range(5)]
                for og in range(4, 6):
                    wts = []
                    for i in range(DI):
                        w = wg.tile([128, 512], bf, tag="wg", name="wg")
                        nc.sync.dma_start(w[:], sa_wqkvT[l, i * 128 : (i + 1) * 128,
                                                         og * 512 : (og + 1) * 512])
                        wts.append(w)
                    vc0 = (og - 4) * 512
                    for kt_i, (k0, kw) in enumerate(KT):
                        p = pk.tile([128, 512], f32, tag="pk", name="pk")
                        for i in range(DI):
                            nc.tensor.matmul(p[:kw, :], hx[b][i][:, k0 : k0 + kw], wts[i][:],
                                             start=(i == 0), stop=False)
                        nc.tensor.matmul(p[:kw, :], ones_b[:, :kw],
                                         bvr[:, vc0 : vc0 + 512],
                                         start=False, stop=True)
                        nc.scalar.copy(vsb[kt_i][:kw, vc0 : vc0 + 512], p[:kw, :])
                attn = []
                for h in range(H):
                    pts = []
                    for kt_i, (k0, kw) in enumerate(KT):
                        bt = bp.tile([128, T], f32, tag="bp", name="bp")
                        nc.sync.dma_start(bt[:kw, :], biasT[h, k0 : k0 + kw, :])
                        sx = f6.tile([128, T], f32, tag="f6", name="f6")
                        for c0, cw in TCH:
                            p = pk.tile([128, 512], f32, tag="pk", name="pk")
                            nc.tensor.matmul(p[:kw, :cw], ksb[h][:, k0 : k0 + kw],
                                             qsb[h][:, c0 : c0 + cw], start=True, stop=True)
                            nc.vector.tensor_tensor(sx[:kw, c0 : c0 + cw], p[:kw, :cw],
                                                    bt[:kw, c0 : c0 + cw], OP.add)
                        pt = w6.tile([128, TK], bf, tag="w6", name="w6")
                        if kw < 128:
                            nc.vector.memset(pt[:, :T], 0.0)
                        nc.scalar.activation(pt[:kw, :T], sx[:kw, :], AF.Exp)
                        pts.append(pt)
                    dn = f6.tile([128, T], f32, tag="f6", name="f6")
                    nc.vector.tensor_tensor(dn[:], pts[0][:, :T], pts[1][:, :T], OP.add)
                    for kt_i in range(2, 5):
                        nc.vector.tensor_tensor(dn[:], dn[:], pts[kt_i][:, :T], OP.add)
                    dsum = f6.tile([128, T], f32, tag="f6", name="f6")
                    nc.gpsimd.partition_all_reduce(dsum[:], dn[:], channels=128, reduce_op=RED.add)
                    rb = f6.tile([128, T], f32, tag="f6", name="f6")
                    nc.vector.reciprocal(rb[:], dsum[:])
                    at = w6.tile([128, TK], bf, tag="w6", name="w6")
                    for c0, cw in TCH:
                        p = pk.tile([128, 512], f32, tag="pk", name="pk")
                        for kt_i, (k0, kw) in enumerate(KT):
                            nc.tensor.matmul(p[:, :cw], vsb[kt_i][:kw, h * 128 : (h + 1) * 128],
                                             pts[kt_i][:kw, c0 : c0 + cw],
                                             start=(kt_i == 0), stop=(kt_i == 4))
                        nc.vector.tensor_tensor(at[:, c0 : c0 + cw], p[:, :cw],
                                                rb[:, c0 : c0 + cw], OP.mult)
                    attn.append(at)
                proj_res(b, sa_woT[l], sa_bo_t[:], attn, 0)
                hq = [hqp.tile([128, TK], bf, tag="hqp", name="hqp") for _ in range(DI)]
                layernorm(b, lng[0][:], lnb[0][:], hq, 0)

                # ================= cross-attention =================
                mx = []
                for i in range(DI):
                    m_ = mxp.tile([128, TK], bf, tag="mxp", name="mxp")
                    nc.sync.dma_start(m_[:, 1:], memT[b, i * 128 : (i + 1) * 128, :])
                    nc.vector.tensor_copy(m_[:, 0:1], adp[i][:, b : b + 1])
                    mx.append(m_)
                ca_attn = []
                for hg in range(2):
                    wq_b = []
                    wk_b = []
                    for i in range(DI):
                        wq_ = wg.tile([128, 512], bf, tag="wg", name="wg")
                        nc.sync.dma_start(wq_[:], ca_wqkvT[l, i * 128 : (i + 1) * 128,
                                                           hg * 512 : hg * 512 + 512])
                        wq_b.append(wq_)
                        wk_ = wg.tile([128, 512], bf, tag="wg", name="wg")
                        nc.sync.dma_start(wk_[:], ca_wqkvT[l, i * 128 : (i + 1) * 128,
                                                           D + hg * 512 : D + hg * 512 + 512])
                        wk_b.append(wk_)
                    qs = []
                    ks = []
                    for hh in range(4):
                        h = hg * 4 + hh
                        osl = slice(hh * 128, (hh + 1) * 128)
                        qh = w6.tile([128, TK], bf, tag="w6", name="w6")
                        for c0, cw in TCH:
                            p = pk.tile([128, 512], f32, tag="pk", name="pk")
                            for i in range(DI):
                                nc.tensor.matmul(p[:, :cw], wq_b[i][:, osl],
                                                 hq[i][:, c0 : c0 + cw],
                                                 start=(i == 0), stop=(i == DI - 1))
                            nc.scalar.activation(qh[:, c0 : c0 + cw], p[:, :cw], AF.Identity,
                                                 bias=ca_bq[:, h : h + 1], scale=INV)
                        qs.append(qh)
                        kh = w6.tile([128, TK], bf, tag="w6", name="w6")
                        for c0, cw in KCH:
                            p = pk.tile([128, 512], f32, tag="pk", name="pk")
                            for i in range(DI):
                                nc.tensor.matmul(p[:, :cw], wk_b[i][:, osl],
                                                 mx[i][:, c0 : c0 + cw],
                                                 start=(i == 0), stop=(i == DI - 1))
                            nc.scalar.activation(kh[:, c0 : c0 + cw], p[:, :cw], AF.Identity,
                                                 bias=ca_bq[:, DI + h : DI + h + 1])
                        ks.append(kh)
                    wv_b = []
                    for i in range(DI):
                        wv_ = wg.tile([128, 512], bf, tag="wg", name="wg")
                        nc.sync.dma_start(wv_[:], ca_wqkvT[l, i * 128 : (i + 1) * 128,
                                                           2 * D + hg * 512 : 2 * D + hg * 512 + 512])
                        wv_b.append(wv_)
                    for hh in range(4):
                        h = hg * 4 + hh
                        osl = slice(hh * 128, (hh + 1) * 128)
                        qh = qs[hh]
                        kh = ks[hh]
                        vh = w6.tile([128, TK], bf, tag="w6", name="w6")
                        for c0, cw in KCH:
                            p = pk.tile([128, 512], f32, tag="pk", name="pk")
                            for i in range(DI):
                                nc.tensor.matmul(p[:, :cw], wv_b[i][:, osl],
                                                 mx[i][:, c0 : c0 + cw],
                                                 start=(i == 0), stop=(i == DI - 1))
                            nc.scalar.activation(vh[:, c0 : c0 + cw], p[:, :cw], AF.Identity,
                                                 bias=ca_bq[:, 2 * DI + h : 2 * DI + h + 1])
                        ka_f = f6.tile([128, 1], f32, tag="ka_f", name="ka_f")
                        nc.vector.tensor_copy(ka_f[:], kh[:, 0:1])
                        va_f = f6.tile([128, 1], f32, tag="va_f", name="va_f")
                        nc.vector.tensor_copy(va_f[:], vh[:, 0:1])
                        qk = f6.tile([128, T], f32, tag="f6", name="f6")
                        nc.vector.tensor_tensor(qk[:], qh[:, :T], kh[:, 1:], OP.mult)
                        smb = f6.tile([128, T], f32, tag="f6", name="f6")
                        nc.gpsimd.partition_all_reduce(smb[:], qk[:], channels=128, reduce_op=RED.add)
                        nc.vector.tensor_scalar_mul(qk[:], qh[:, :T], ka_f[:])
                        sab = f6.tile([128, T], f32, tag="f6", name="f6")
                        nc.gpsimd.partition_all_reduce(sab[:], qk[:], channels=128, reduce_op=RED.add)
                        dd = f6.tile([128, T], f32, tag="f6", name="f6")
                        nc.vector.tensor_tensor(dd[:], smb[:], sab[:], OP.subtract)
                        wm = f6.tile([128, T], f32, tag="f6", name="f6")
                        nc.scalar.activation(wm[:], dd[:], AF.Sigmoid)
                        at = w6.tile([128, TK], bf, tag="w6", name="w6")
                        t1 = f6.tile([128, T], f32, tag="f6", name="f6")
                        nc.vector.tensor_scalar_sub(t1[:], vh[:, 1:], va_f[:])
                        nc.vector.tensor_tensor(t1[:], t1[:], wm[:], OP.mult)
                        nc.vector.tensor_scalar_add(at[:, :T], t1[:], va_f[:])
                        ca_attn.append(at)
                proj_res(b, ca_woT[l], ca_bo_t[:], ca_attn, 0)
                hq = [hqp.tile([128, TK], bf, tag="hqp", name="hqp") for _ in range(DI)]
                layernorm(b, lng[1][:], lnb[1][:], hq, 0)

                # ================= FFN =================
                xacc = [f6.tile([128, T], f32, tag="f6", name="f6") for _ in range(DI)]
                for half in range(2):
                    ffa = []
                    for og in range(4):
                        go = half * 4 + og
                        wts = []
                        for i in range(DI):
                            w = wg.tile([128, 512], bf, tag="wg", name="wg")
                            nc.sync.dma_start(w[:], ff_w1T[l, i * 128 : (i + 1) * 128,
                                                          go * 512 : (go + 1) * 512])
                            wts.append(w)
                        for ot in range(4):
                            o = go * 4 + ot
                            dst = fa.tile([128, TK], bf, tag="fa", name="fa")
                            for c0, cw in TCH:
                                p = pk.tile([128, 512], f32, tag="pk", name="pk")
                                for i in range(DI):
                                    nc.tensor.matmul(p[:, :cw], wts[i][:, ot * 128 : (ot + 1) * 128],
                                                     hq[i][:, c0 : c0 + cw],
                                                     start=(i == 0), stop=(i == DI - 1))
                                nc.scalar.activation(dst[:, c0 : c0 + cw], p[:, :cw], AF.Relu,
                                                     bias=f_b1[:, o : o + 1])
                            ffa.append(dst)
                    for og2 in range(4):
                        w2t = []
                        for ii in range(16):
                            i_t = half * 16 + ii
                            w = wg.tile([128, 512], bf, tag="wg", name="wg")
                            nc.sync.dma_start(w[:, :256], ff_w2T[l, i_t * 128 : (i_t + 1) * 128,
                                                                 og2 * 256 : (og2 + 1) * 256])
                            w2t.append(w)
                        for o2t in range(2):
                            o = og2 * 2 + o2t
                            for c0, cw in TCH:
                                p = pk.tile([128, 512], f32, tag="pk", name="pk")
                                for ii in range(16):
                                    nc.tensor.matmul(p[:, :cw],
                                                     w2t[ii][:, o2t * 128 : o2t * 128 + 128],
                                                     ffa[ii][:, c0 : c0 + cw],
                                                     start=(ii == 0), stop=(ii == 15))
                                if half == 0:
                                    nc.vector.scalar_tensor_tensor(
                                        xacc[o][:, c0 : c0 + cw], p[:, :cw],
                                        f_b2[:, o : o + 1], hf[b][o][:, c0 : c0 + cw],
                                        OP.add, OP.add)
                                else:
                                    nc.vector.tensor_tensor(hf[b][o][:, c0 : c0 + cw],
                                                            p[:, :cw], xacc[o][:, c0 : c0 + cw],
                                                            OP.add)
                layernorm(b, lng[2][:], lnb[2][:], hx[b], 1)
                for i in range(DI):
                    nc.vector.tensor_copy(hx[b][i][:, 0:1], adp[i][:, b : b + 1])

            # ---------- output projection ----------
            wo_t = []
            for i in range(DI):
                w = wg.tile([128, IN], bf, tag="wgout", name="wgout")
                nc.sync.dma_start(w[:], w_outT[i * 128 : (i + 1) * 128, :])
                wo_t.append(w)
            ot_ = sm.tile([IN, T], f32, tag=f"osb{b}", name=f"osb{b}")
            for c0, cw in TCH:
                p = pk.tile([128, 512], f32, tag="pk", name="pk")
                for i in range(DI):
                    nc.tensor.matmul(p[:IN, :cw], wo_t[i][:], hx[b][i][:, 1 + c0 : 1 + c0 + cw],
                                     start=(i == 0), stop=(i == DI - 1))
                nc.scalar.activation(ot_[:, c0 : c0 + cw], p[:IN, :cw], AF.Identity, bias=bo_t[:])
            nc.sync.dma_start(out_d[b], ot_[:])

        for _pool in (pk, sm, bp, wg, f6, vs, mxp, hqp, fa, w6, res):
            _pool.release()

    nc.compile()
    return nc


def _prep_host(inputs):
    """Build the 8 per-core input maps from full inputs."""
    f32 = np.float32

    def b16(a):
        return np.ascontiguousarray(np.asarray(a, f32)).astype(bf16np)

    def tiled(vec, n):          # [n*128] -> [128, n] (col j = tile j)
        return np.ascontiguousarray(np.asarray(vec, f32).reshape(n, 128).T)

    x = np.asarray(inputs["x"], f32)
    memory = np.asarray(inputs["memory"], f32)
    ts = np.asarray(inputs["timesteps"])
    pe = np.asarray(inputs["pe"], f32)
    alibi = np.asarray(inputs["alibi"], f32)

    half = D // 2
    expo = np.exp(-math.log(10000.0) * np.arange(half, dtype=f32) / (half - 1.0))
    efm = np.concatenate([expo, expo]) / (2 * np.pi)
    phs = np.concatenate([np.zeros(half, f32), np.full(half, 0.25, f32)])

    biasT = np.concatenate([np.zeros((H, T, 1), f32), alibi], axis=-1).transpose(0, 2, 1)
    biasT = np.ascontiguousarray(biasT)

    qkv_bias = {}
    for nm in ("sa", "ca"):
        bq = np.asarray(inputs[f"{nm}_bqkv"], f32).copy()      # [L, 3D]
        bq[:, :D] *= INV                                       # pre-scale q bias
        qkv_bias[nm] = np.stack([np.stack([tiled(bq[l, k * 128 : (k + 1) * 128 * 0 + (k + 1) * 128], 1)[:, 0]
                                           for k in range(3 * DI)], axis=1)
                                 for l in range(L)])           # [L,128,24]

    common = {
        "tsf": None, "xT": None, "memT": None,
        "efm": tiled(efm, DI), "phs": tiled(phs, DI),
        "peT": np.ascontiguousarray(pe.T + np.asarray(inputs["b_in"], f32)[:, None]),
        "w_inT": b16(np.asarray(inputs["W_in"], f32).T),
        "te_w1T": b16(np.asarray(inputs["te_W1"], f32).T),
        "te_w2T": b16(np.asarray(inputs["te_W2"], f32).T),
        "te_b1t": tiled(inputs["te_b1"], DI),
        "te_b2t": tiled(inputs["te_b2"], DI),
        "sa_wqkvT": b16(np.asarray(inputs["sa_Wqkv"], f32).transpose(0, 2, 1)),
        "sa_bqkvt": qkv_bias["sa"],
        "sa_bvrow": b16(np.asarray(inputs["sa_bqkv"], f32)[:, 2 * D :][:, None, :]),
        "sa_woT": b16(np.asarray(inputs["sa_Wo"], f32).transpose(0, 2, 1)),
        "sa_bot": np.stack([tiled(np.asarray(inputs["sa_bo"], f32)[l], DI) for l in range(L)]),
        "ca_wqkvT": b16(np.asarray(inputs["ca_Wqkv"], f32).transpose(0, 2, 1)),
        "ca_bqkvt": qkv_bias["ca"],
        "ca_woT": b16(np.asarray(inputs["ca_Wo"], f32).transpose(0, 2, 1)),
        "ca_bot": np.stack([tiled(np.asarray(inputs["ca_bo"], f32)[l], DI) for l in range(L)]),
        "ff_w1T": b16(np.asarray(inputs["ff_W1"], f32).transpose(0, 2, 1)),
        "ff_b1t": np.stack([tiled(np.asarray(inputs["ff_b1"], f32)[l], DFI) for l in range(L)]),
        "ff_w2T": b16(np.asarray(inputs["ff_W2"], f32).transpose(0, 2, 1)),
        "ff_b2t": np.stack([tiled(np.asarray(inputs["ff_b2"], f32)[l], DI) for l in range(L)]),
        "lngt": np.stack([np.stack([tiled(np.asarray(inputs[f"ln{k+1}_g"], f32)[l], DI)
                                    for k in range(3)]) for l in range(L)]),
        "lnbt": np.stack([np.stack([tiled(np.asarray(inputs[f"ln{k+1}_b"], f32)[l], DI)
                                    for k in range(3)]) for l in range(L)]),
        "biasT": biasT,
        "w_outT": b16(np.asarray(inputs["W_out"], f32).T),
        "b_out": np.asarray(inputs["b_out"], f32)[:, None],
    }

    in_maps = []
    for c in range(NC):
        b0 = c * BC
        m = dict(common)
        m["xT"] = b16(x[b0 : b0 + BC].transpose(0, 2, 1))
        m["memT"] = b16(memory[b0 : b0 + BC].transpose(0, 2, 1))
        m["tsf"] = np.asarray(ts[b0 : b0 + BC], f32)[None, :]
        in_maps.append(m)
    return in_maps


def kernel(**inputs):
    from concourse.bass_utils import run_bass_kernel_spmd

    if "nc" not in _cache:
        _cache["nc"] = _build()
    nc = _cache["nc"]
    in_maps = _prep_host(inputs)
    res = run_bass_kernel_spmd(nc, in_maps, core_ids=list(range(NC)))
    out = np.empty((B, T, IN), np.float32)
    for c in range(NC):
        out[c * BC : (c + 1) * BC] = res.results[c]["out"].transpose(0, 2, 1)
    return out



# revision 6
# speedup vs baseline: 1.5516x; 1.5516x over previous
import math
import sys

sys.path.insert(0, "/opt/trn_rl_repo")

import numpy as np
import ml_dtypes

bf16np = ml_dtypes.bfloat16

# ---------------- problem constants (hardcoded; kernel.py must be self-contained) ----
B, T, S, D, H, L, DFF, IN, PERIOD = 16, 600, 600, 1024, 8, 8, 4096, 52, 25
HD = D // H          # 128
NC = 8               # cores
BC = B // NC         # 2 batches per core
DI = D // 128        # 8 i-tiles
DFI = DFF // 128     # 32
INV = 1.0 / math.sqrt(HD)
TK = 1 + T           # 601 keys (adapter + T)
# chunks of the token dim (>=256 wide keeps LDWEIGHTS hidden; <=512 fits a psum bank)
TCH = [(0, 300), (300, 300)]
KCH = [(0, 301), (301, 300)]         # 601-wide
KT = [(0, 128), (128, 128), (256, 128), (384, 128), (512, 89)]  # key tiles of 601
SLOPES = [0.5 ** (h + 1) for h in range(H)]

_cache = {}


def _build():
    """Build the per-core Bass graph (SPMD; same program all 8 cores)."""
    from concourse import bacc, mybir
    import concourse.bass as bass
    import concourse.tile as tile

    f32 = mybir.dt.float32
    bf = mybir.dt.bfloat16
    i32 = mybir.dt.int32
    AF = mybir.ActivationFunctionType
    OP = mybir.AluOpType

    nc = bacc.Bacc("TRN2", target_bir_lowering=False, debug=False, num_devices=NC)

    def din(name, shape, dt=f32):
        return nc.dram_tensor(name, shape, dt, kind="ExternalInput").ap()

    # ---- DRAM inputs (host-prepped layouts; *_c = tile-contiguous) ----
    xT = din("xT", [BC, IN, T], bf)              # x transposed, bf16
    memT_c = din("memT_c", [BC, DI, 128, T], bf)
    tsf = din("tsf", [1, BC])                    # timesteps as f32
    efm = din("efm", [128, DI])                  # e/(2pi) tiled per i-tile col
    phs = din("phs", [128, DI])                  # phase (0 / .25)
    peT_c = din("peT_c", [DI, 128, T])           # pe.T + b_in  (f32)
    w_inT = din("w_inT", [IN, D], bf)
    te_w1T_c = din("te_w1T_c", [DI, 128, D], bf)
    te_w2T_c = din("te_w2T_c", [DI, 128, D], bf)
    te_b1t = din("te_b1t", [128, DI])
    te_b2t = din("te_b2t", [128, DI])
    sa_wqkvT_c = din("sa_wqkvT_c", [L, 3, DI, 128, D], bf)
    sa_bqkvt = din("sa_bqkvt", [L, 128, 3 * DI])  # pre-tiled [128, 24]; q-part prescaled by INV
    sa_bvrow = din("sa_bvrow", [L, 1, D], bf)     # v-bias as row (for ones-MM trick)
    sa_woT_c = din("sa_woT_c", [L, DI, 128, D], bf)
    sa_bot = din("sa_bot", [L, 128, DI])
    ca_wqkvT_c = din("ca_wqkvT_c", [L, 3, DI, 128, D], bf)
    ca_bqkvt = din("ca_bqkvt", [L, 128, 3 * DI])
    ca_woT_c = din("ca_woT_c", [L, DI, 128, D], bf)
    ca_bot = din("ca_bot", [L, 128, DI])
    ff_w1T_c = din("ff_w1T_c", [L, 4, DI, 128, D], bf)
    ff_b1t = din("ff_b1t", [L, 128, DFI])
    ff_w2T_c = din("ff_w2T_c", [L, 2, 16, 128, D], bf)
    ff_b2t = din("ff_b2t", [L, 128, DI])
    lngt = din("lngt", [L, 3, 128, DI])
    lnbt = din("lnbt", [L, 3, 128, DI])
    steps_d = din("steps_d", [5, 128, T], bf)    # alibi steps, kt-tiled [k, q]
    w_outT_c = din("w_outT_c", [DI, 128, IN], bf)
    b_out = din("b_out", [IN, 1])
    out_d = nc.dram_tensor("out", [BC, IN, T], f32, kind="ExternalOutput").ap()

    with tile.TileContext(nc) as tc:
        res = tc.alloc_tile_pool(name="res", bufs=1)      # persistent
        w6 = tc.alloc_tile_pool(name="w6", bufs=26)       # bf16 [128,601] q/k/pt/attn ws
        fa = tc.alloc_tile_pool(name="fa", bufs=17)       # bf16 [128,601] ffa tiles
        hqp = tc.alloc_tile_pool(name="hqp", bufs=10)     # bf16 [128,601] LN targets
        mxp = tc.alloc_tile_pool(name="mxp", bufs=9)      # bf16 [128,601] mem staging
        vs = tc.alloc_tile_pool(name="vs", bufs=6)        # bf16 [128,1024] V tiles
        f6 = tc.alloc_tile_pool(name="f6", bufs=7)        # f32 [128,600] workspace
        sqp = tc.alloc_tile_pool(name="sqp", bufs=8)      # bf16 [128,600] LN squares
        wg = tc.alloc_tile_pool(name="wg", bufs=17)       # bf16 [128,1024] weights
        sm = tc.alloc_tile_pool(name="sm", bufs=1)        # small persistents
        pk = tc.alloc_tile_pool(name="pk", bufs=8, space="PSUM")

        # persistent bf16 residual stream (adapter col 0) - ONE batch at a time
        hb1 = [res.tile([128, TK], bf, tag=f"hb_{i}", name=f"hb_{i}") for i in range(DI)]
        hb = [hb1 for _ in range(BC)]

        ones_b = sm.tile([1, 128], bf, tag="ones_b", name="ones_b")
        nc.vector.memset(ones_b[:], 1.0)
        ones_b128 = sm.tile([128, 128], bf, tag="ones_b128", name="ones_b128")
        nc.vector.memset(ones_b128[:], 1.0)
        ones_f = sm.tile([1, 128], f32, tag="ones_f", name="ones_f")
        nc.vector.memset(ones_f[:], 1.0)
        eft = sm.tile([128, DI], f32, tag="eft", name="eft")
        nc.sync.dma_start(eft[:], efm[:])
        pht = sm.tile([128, DI], f32, tag="pht", name="pht")
        nc.sync.dma_start(pht[:], phs[:])
        tst = sm.tile([1, BC], f32, tag="tst", name="tst")
        eps_t = sm.tile([128, 1], f32, tag="eps_t", name="eps_t")
        nc.vector.memset(eps_t[:], 1e-5)
        nc.sync.dma_start(tst[:], tsf[:])
        adp = [sm.tile([128, BC], bf, tag=f"adp{i}", name=f"adp{i}") for i in range(DI)]  # adapter bf16
        steps_t = [sm.tile([128, T], bf, tag=f"steps{k}", name=f"steps{k}") for k in range(5)]
        for k in range(5):
            nc.sync.dma_start(steps_t[k][:], steps_d[k])

        # ---------- timestep embedding ----------
        ptb = pk.tile([128, BC], f32, tag="pk", name="ptb")
        nc.tensor.matmul(ptb[:], ones_f[:], tst[:], start=True, stop=True)  # t bcast f32
        temb = []
        for i in range(DI):
            y = sm.tile([128, BC], f32, tag=f"y{i}", name=f"y{i}")
            nc.vector.tensor_scalar_mul(y[:], ptb[:], eft[:, i : i + 1])
            nc.vector.tensor_scalar_add(y[:], y[:], pht[:, i : i + 1])
            yi = sm.tile([128, BC], i32, tag=f"yi{i}", name=f"yi{i}")
            nc.vector.tensor_copy(yi[:], y[:])
            yr = sm.tile([128, BC], f32, tag=f"yr{i}", name=f"yr{i}")
            nc.vector.tensor_copy(yr[:], yi[:])
            fr = sm.tile([128, BC], f32, tag=f"fr{i}", name=f"fr{i}")
            nc.vector.tensor_sub(fr[:], y[:], yr[:])
            tb = sm.tile([128, BC], bf, tag=f"tb{i}", name=f"tb{i}")
            nc.scalar.activation(tb[:], fr[:], AF.Sin, scale=2 * math.pi)
            temb.append(tb)

        def mlp1024(wT_c, bt_d, ins, act, outs_tag):
            """[D,D] proj on BC-wide f-major input tiles. Returns 8 bf16 [128,BC] tiles."""
            bt = sm.tile([128, DI], f32, tag=outs_tag + "_b", name=outs_tag + "_b")
            nc.sync.dma_start(bt[:], bt_d[:])
            ws = []
            for i in range(DI):
                w = wg.tile([128, D], bf, tag="wg", name="wg")
                nc.sync.dma_start(w[:], wT_c[i])
                ws.append(w)
            outs = []
            for o in range(DI):
                p = pk.tile([128, BC], f32, tag="pk", name="pmlp")
                for i in range(DI):
                    nc.tensor.matmul(p[:], ws[i][:, o * 128 : (o + 1) * 128], ins[i][:],
                                     start=(i == 0), stop=(i == DI - 1))
                ob = sm.tile([128, BC], bf, tag=f"{outs_tag}{o}", name=f"{outs_tag}{o}")
                nc.scalar.activation(ob[:], p[:], act, bias=bt[:, o : o + 1])
                outs.append(ob)
            return outs

        z1 = mlp1024(te_w1T_c, te_b1t, temb, AF.Silu, "z1")
        z2 = mlp1024(te_w2T_c, te_b2t, z1, AF.Identity, "z2")
        for i in range(DI):
            nc.vector.tensor_copy(adp[i][:], z2[i][:])

        # ---------- helpers ----------
        def layernorm(b, g_ap, b_ap, tgt=None):
            """LN over features of X held in hb[b][:, 1:] (in place, bf16);
            optional bf16 copy to tgt[o][:, :T]. Sums via ones-matmul reduction."""
            sq = []
            for o in range(DI):
                s = sqp.tile([128, T], bf, tag="sqp", name="sqp")
                nc.scalar.activation(s[:], hb[b][o][:, 1:], AF.Square)
                sq.append(s)
            m = f6.tile([128, T], f32, tag="f6", name="f6")
            rstd = f6.tile([128, T], f32, tag="f6", name="f6")
            mr = f6.tile([128, T], f32, tag="f6", name="f6")
            for c0, cw in TCH:
                pS = pk.tile([128, 512], f32, tag="pk", name="pk")
                for o in range(DI):
                    nc.tensor.matmul(pS[:, :cw], ones_b128[:],
                                     hb[b][o][:, 1 + c0 : 1 + c0 + cw],
                                     start=(o == 0), stop=(o == DI - 1))
                pS2 = pk.tile([128, 512], f32, tag="pk", name="pk")
                for o in range(DI):
                    nc.tensor.matmul(pS2[:, :cw], ones_b128[:], sq[o][:, c0 : c0 + cw],
                                     start=(o == 0), stop=(o == DI - 1))
                nc.vector.tensor_scalar_mul(m[:, c0 : c0 + cw], pS[:, :cw], 1.0 / D)
                m2 = f6.tile([128, T], f32, tag="f6", name="f6")
                nc.vector.tensor_tensor(m2[:, :cw], m[:, c0 : c0 + cw], m[:, c0 : c0 + cw], OP.mult)
                var = f6.tile([128, T], f32, tag="f6", name="f6")
                nc.vector.scalar_tensor_tensor(var[:, :cw], pS2[:, :cw], 1.0 / D, m2[:, :cw],
                                               OP.mult, OP.subtract)
                lv = f6.tile([128, T], f32, tag="f6", name="f6")
                nc.scalar.activation(lv[:, :cw], var[:, :cw], AF.Ln, bias=eps_t[:])
                nc.scalar.activation(rstd[:, c0 : c0 + cw], lv[:, :cw], AF.Exp, scale=-0.5)
                nc.vector.tensor_tensor(mr[:, c0 : c0 + cw], m[:, c0 : c0 + cw],
                                        rstd[:, c0 : c0 + cw], OP.mult)
            for o in range(DI):
                t1 = f6.tile([128, T], f32, tag="f6", name="f6")
                nc.vector.tensor_tensor(t1[:], hb[b][o][:, 1:], rstd[:], OP.mult)
                nc.vector.tensor_tensor(t1[:], t1[:], mr[:], OP.subtract)
                nc.scalar.activation(hb[b][o][:, 1:], t1[:], AF.Identity,
                                     bias=b_ap[:, o : o + 1], scale=g_ap[:, o : o + 1])
                if tgt is not None:
                    nc.vector.tensor_copy(tgt[o][:, :T], hb[b][o][:, 1:])

        def proj_res(b, wT_c_l, bot_ap, rhs_tiles):
            """out-proj [D,D] + bias + residual into hf[b] (X pre-LN)."""
            ws = []
            for i in range(DI):
                w = wg.tile([128, D], bf, tag="wg", name="wg")
                nc.sync.dma_start(w[:], wT_c_l[i])
                ws.append(w)
            for o in range(DI):
                for c0, cw in TCH:
                    p = pk.tile([128, 512], f32, tag="pk", name="pk")
                    for i in range(DI):
                        nc.tensor.matmul(p[:, :cw], ws[i][:, o * 128 : (o + 1) * 128],
                                         rhs_tiles[i][:, c0 : c0 + cw],
                                         start=(i == 0), stop=(i == DI - 1))
                    nc.vector.scalar_tensor_tensor(hb[b][o][:, 1 + c0 : 1 + c0 + cw], p[:, :cw],
                                                   bot_ap[:, o : o + 1],
                                                   hb[b][o][:, 1 + c0 : 1 + c0 + cw],
                                                   OP.add, OP.add)

        # per-layer bias tiles (re-DMAed each (b, l))
        sa_bq = sm.tile([128, 3 * DI], f32, tag="sa_bq", name="sa_bq")
        ca_bq = sm.tile([128, 3 * DI], f32, tag="ca_bq", name="ca_bq")
        sa_bo_t = sm.tile([128, DI], f32, tag="sa_bo_t", name="sa_bo_t")
        ca_bo_t = sm.tile([128, DI], f32, tag="ca_bo_t", name="ca_bo_t")
        f_b1 = sm.tile([128, DFI], f32, tag="f_b1", name="f_b1")
        f_b2 = sm.tile([128, DI], f32, tag="f_b2", name="f_b2")
        lng = [sm.tile([128, DI], f32, tag=f"lng{k}", name=f"lng{k}") for k in range(3)]
        lnb = [sm.tile([128, DI], f32, tag=f"lnb{k}", name=f"lnb{k}") for k in range(3)]
        bvr = sm.tile([1, D], bf, tag="bvr", name="bvr")
        bo_t = sm.tile([IN, 1], f32, tag="bo_t", name="bo_t")
        nc.sync.dma_start(bo_t[:], b_out[:])

        # ================= batch-serial main =================
        for b in range(BC):
            # ---------- input projection + pe ----------
            xb = sm.tile([IN, T], bf, tag="xb", name="xb")
            nc.sync.dma_start(xb[:], xT[b])
            w_in_t = sm.tile([IN, D], bf, tag="w_in_t", name="w_in_t")
            nc.sync.dma_start(w_in_t[:], w_inT[:])
            for o in range(DI):
                pe_t = f6.tile([128, T], f32, tag="f6", name="f6")
                nc.sync.dma_start(pe_t[:], peT_c[o])
                for c0, cw in TCH:
                    p = pk.tile([128, 512], f32, tag="pk", name="pk")
                    nc.tensor.matmul(p[:, :cw], w_in_t[:, o * 128 : (o + 1) * 128],
                                     xb[:, c0 : c0 + cw], start=True, stop=True)
                    nc.vector.tensor_tensor(hb[b][o][:, 1 + c0 : 1 + c0 + cw], p[:, :cw],
                                            pe_t[:, c0 : c0 + cw], OP.add)
                nc.vector.tensor_copy(hb[b][o][:, 0:1], adp[o][:, b : b + 1])

            for l in range(L):
                nc.sync.dma_start(sa_bq[:], sa_bqkvt[l])
                nc.sync.dma_start(ca_bq[:], ca_bqkvt[l])
                nc.sync.dma_start(sa_bo_t[:], sa_bot[l])
                nc.sync.dma_start(ca_bo_t[:], ca_bot[l])
                nc.sync.dma_start(f_b1[:], ff_b1t[l])
                nc.sync.dma_start(f_b2[:], ff_b2t[l])
                for k in range(3):
                    nc.sync.dma_start(lng[k][:], lngt[l, k])
                    nc.sync.dma_start(lnb[k][:], lnbt[l, k])
                nc.sync.dma_start(bvr[:], sa_bvrow[l])

                # ================= self-attention =================
                def sa_proj(mat, src_off, chunks, bias_off, scale):
                    ws = []
                    for i in range(DI):
                        w = wg.tile([128, D], bf, tag="wg", name="wg")
                        nc.sync.dma_start(w[:], sa_wqkvT_c[l, mat, i])
                        ws.append(w)
                    outs = []
                    for ot in range(DI):
                        dst = w6.tile([128, TK], bf, tag="w6", name="w6")
                        for c0, cw in chunks:
                            p = pk.tile([128, 512], f32, tag="pk", name="pk")
                            for i in range(DI):
                                nc.tensor.matmul(p[:, :cw], ws[i][:, ot * 128 : (ot + 1) * 128],
                                                 hb[b][i][:, src_off + c0 : src_off + c0 + cw],
                                                 start=(i == 0), stop=(i == DI - 1))
                            nc.scalar.activation(dst[:, c0 : c0 + cw], p[:, :cw], AF.Identity,
                                                 bias=sa_bq[:, bias_off + ot : bias_off + ot + 1],
                                                 scale=scale)
                        outs.append(dst)
                    return outs

                qsb = sa_proj(0, 1, TCH, 0, INV)
                ksb = sa_proj(1, 0, KCH, DI, 1.0)
                # V transposed ([key, head*hd]) via hx-stationary matmuls
                vw = []
                for i in range(DI):
                    w = wg.tile([128, D], bf, tag="wg", name="wg")
                    nc.sync.dma_start(w[:], sa_wqkvT_c[l, 2, i])
                    vw.append(w)
                vsb = [vs.tile([128, 1024], bf, tag="vs", name="vs") for _ in range(5)]
                for vc in range(2):
                    vc0 = vc * 512
                    for kt_i, (k0, kwd) in enumerate(KT):
                        p = pk.tile([128, 512], f32, tag="pk", name="pk")
                        for i in range(DI):
                            nc.tensor.matmul(p[:kwd, :], hb[b][i][:, k0 : k0 + kwd],
                                             vw[i][:, vc0 : vc0 + 512],
                                             start=(i == 0), stop=False)
                        nc.tensor.matmul(p[:kwd, :], ones_b[:, :kwd],
                                         bvr[:, vc0 : vc0 + 512], start=False, stop=True)
                        nc.scalar.copy(vsb[kt_i][:kwd, vc0 : vc0 + 512], p[:kwd, :])

                attn = []
                for h in range(H):
                    pts = []
                    for kt_i, (k0, kwd) in enumerate(KT):
                        sx = f6.tile([128, T], f32, tag="f6", name="f6")
                        for c0, cw in TCH:
                            p = pk.tile([128, 512], f32, tag="pk", name="pk")
                            nc.tensor.matmul(p[:kwd, :cw], ksb[h][:, k0 : k0 + kwd],
                                             qsb[h][:, c0 : c0 + cw], start=True, stop=True)
                            nc.vector.scalar_tensor_tensor(sx[:kwd, c0 : c0 + cw],
                                                           steps_t[kt_i][:kwd, c0 : c0 + cw],
                                                           -SLOPES[h], p[:kwd, :cw],
                                                           OP.mult, OP.add)
                        pt = w6.tile([128, TK], bf, tag="w6", name="w6")
                        nc.scalar.activation(pt[:kwd, :T], sx[:kwd, :], AF.Exp)
                        pts.append(pt)
                    rb = f6.tile([128, T], f32, tag="f6", name="f6")
                    for c0, cw in TCH:
                        pd = pk.tile([128, 512], f32, tag="pk", name="pk")
                        for kt_i, (k0, kwd) in enumerate(KT):
                            nc.tensor.matmul(pd[:, :cw], ones_b128[:kwd, :],
                                             pts[kt_i][:kwd, c0 : c0 + cw],
                                             start=(kt_i == 0), stop=(kt_i == 4))
                        lv = f6.tile([128, T], f32, tag="f6", name="f6")
                        nc.scalar.activation(lv[:, :cw], pd[:, :cw], AF.Ln)
                        nc.scalar.activation(rb[:, c0 : c0 + cw], lv[:, :cw], AF.Exp, scale=-1.0)
                    at = w6.tile([128, TK], bf, tag="w6", name="w6")
                    for c0, cw in TCH:
                        p = pk.tile([128, 512], f32, tag="pk", name="pk")
                        for kt_i, (k0, kwd) in enumerate(KT):
                            nc.tensor.matmul(p[:, :cw], vsb[kt_i][:kwd, h * 128 : (h + 1) * 128],
                                             pts[kt_i][:kwd, c0 : c0 + cw],
                                             start=(kt_i == 0), stop=(kt_i == 4))
                        nc.vector.tensor_tensor(at[:, c0 : c0 + cw], p[:, :cw],
                                                rb[:, c0 : c0 + cw], OP.mult)
                    attn.append(at)
                proj_res(b, sa_woT_c[l], sa_bo_t[:], attn)
                layernorm(b, lng[0][:], lnb[0][:])

                # ================= cross-attention =================
                mx = []
                for i in range(DI):
                    m_ = mxp.tile([128, TK], bf, tag="mxp", name="mxp")
                    nc.sync.dma_start(m_[:, 1:], memT_c[b, i])
                    nc.vector.tensor_copy(m_[:, 0:1], adp[i][:, b : b + 1])
                    mx.append(m_)

                def ca_proj(mat, src_tiles, src_off, chunks, bias_off, scale):
                    ws = []
                    for i in range(DI):
                        w = wg.tile([128, D], bf, tag="wg", name="wg")
                        nc.sync.dma_start(w[:], ca_wqkvT_c[l, mat, i])
                        ws.append(w)
                    outs = []
                    for ot in range(DI):
                        dst = w6.tile([128, TK], bf, tag="w6", name="w6")
                        for c0, cw in chunks:
                            p = pk.tile([128, 512], f32, tag="pk", name="pk")
                            for i in range(DI):
                                nc.tensor.matmul(p[:, :cw], ws[i][:, ot * 128 : (ot + 1) * 128],
                                                 src_tiles[i][:, src_off + c0 : src_off + c0 + cw],
                                                 start=(i == 0), stop=(i == DI - 1))
                            nc.scalar.activation(dst[:, c0 : c0 + cw], p[:, :cw], AF.Identity,
                                                 bias=ca_bq[:, bias_off + ot : bias_off + ot + 1],
                                                 scale=scale)
                        outs.append(dst)
                    return outs

                # k/v first: they depend only on memory, so they overlap LN1
                ks_ca = ca_proj(1, mx, 0, KCH, DI, 1.0)
                vs_ca = ca_proj(2, mx, 0, KCH, 2 * DI, 1.0)
                qs_ca = ca_proj(0, hb[b], 1, TCH, 0, INV)

                ca_attn = []
                for h in range(H):
                    kh, vh, qh = ks_ca[h], vs_ca[h], qs_ca[h]
                    ka = f6.tile([128, 1], f32, tag="ka", name="ka")
                    nc.vector.tensor_copy(ka[:], kh[:, 0:1])
                    va = f6.tile([128, 1], f32, tag="ka", name="va")
                    nc.vector.tensor_copy(va[:], vh[:, 0:1])
                    kd = w6.tile([128, TK], bf, tag="w6", name="w6")
                    nc.vector.tensor_scalar_sub(kd[:, :T], kh[:, 1:], ka[:])
                    e = w6.tile([128, TK], bf, tag="w6", name="w6")
                    nc.vector.tensor_tensor(e[:, :T], qh[:, :T], kd[:, :T], OP.mult)
                    wm = w6.tile([128, TK], bf, tag="w6", name="w6")
                    for c0, cw in TCH:
                        pd = pk.tile([128, 512], f32, tag="pk", name="pk")
                        nc.tensor.matmul(pd[:, :cw], ones_b128[:], e[:, c0 : c0 + cw],
                                         start=True, stop=True)
                        nc.scalar.activation(wm[:, c0 : c0 + cw], pd[:, :cw], AF.Sigmoid)
                    vd = w6.tile([128, TK], bf, tag="w6", name="w6")
                    nc.vector.tensor_scalar_sub(vd[:, :T], vh[:, 1:], va[:])
                    at = w6.tile([128, TK], bf, tag="w6", name="w6")
                    nc.vector.tensor_tensor(at[:, :T], vd[:, :T], wm[:, :T], OP.mult)
                    nc.vector.tensor_scalar_add(at[:, :T], at[:, :T], va[:])
                    ca_attn.append(at)
                proj_res(b, ca_woT_c[l], ca_bo_t[:], ca_attn)
                hq = [hqp.tile([128, TK], bf, tag="hqp", name="hqp") for _ in range(DI)]
                layernorm(b, lng[1][:], lnb[1][:], hq)

                # ================= FFN =================
                for half in range(2):
                    ffa = []
                    for g2 in range(2):
                        gi = half * 2 + g2
                        w1s = []
                        for i in range(DI):
                            w = wg.tile([128, D], bf, tag="wg", name="wg")
                            nc.sync.dma_start(w[:], ff_w1T_c[l, gi, i])
                            w1s.append(w)
                        for ot in range(DI):
                            o = gi * DI + ot
                            dst = fa.tile([128, TK], bf, tag="fa", name="fa")
                            for c0, cw in TCH:
                                p = pk.tile([128, 512], f32, tag="pk", name="pk")
                                for i in range(DI):
                                    nc.tensor.matmul(p[:, :cw], w1s[i][:, ot * 128 : (ot + 1) * 128],
                                                     hq[i][:, c0 : c0 + cw],
                                                     start=(i == 0), stop=(i == DI - 1))
                                nc.scalar.activation(dst[:, c0 : c0 + cw], p[:, :cw], AF.Relu,
                                                     bias=f_b1[:, o : o + 1])
                            ffa.append(dst)
                    w2s = []
                    for ii in range(16):
                        w = wg.tile([128, D], bf, tag="wg", name="wg")
                        nc.sync.dma_start(w[:], ff_w2T_c[l, half, ii])
                        w2s.append(w)
                    for o in range(DI):
                        for c0, cw in TCH:
                            p = pk.tile([128, 512], f32, tag="pk", name="pk")
                            for ii in range(16):
                                nc.tensor.matmul(p[:, :cw], w2s[ii][:, o * 128 : (o + 1) * 128],
                                                 ffa[ii][:, c0 : c0 + cw],
                                                 start=(ii == 0), stop=(ii == 15))
                            if half == 0:
                                nc.vector.scalar_tensor_tensor(
                                    hb[b][o][:, 1 + c0 : 1 + c0 + cw], p[:, :cw],
                                    f_b2[:, o : o + 1], hb[b][o][:, 1 + c0 : 1 + c0 + cw],
                                    OP.add, OP.add)
                            else:
                                nc.vector.tensor_tensor(hb[b][o][:, 1 + c0 : 1 + c0 + cw],
                                                        p[:, :cw],
                                                        hb[b][o][:, 1 + c0 : 1 + c0 + cw],
                                                        OP.add)
                layernorm(b, lng[2][:], lnb[2][:])

            # ---------- output projection ----------
            wo_t = []
            for i in range(DI):
                w = wg.tile([128, IN], bf, tag="wgout", name="wgout")
                nc.sync.dma_start(w[:], w_outT_c[i])
                wo_t.append(w)
            ot_ = sm.tile([IN, T], f32, tag=f"osb{b}", name=f"osb{b}")
            for c0, cw in TCH:
                p = pk.tile([128, 512], f32, tag="pk", name="pk")
                for i in range(DI):
                    nc.tensor.matmul(p[:IN, :cw], wo_t[i][:], hb[b][i][:, 1 + c0 : 1 + c0 + cw],
                                     start=(i == 0), stop=(i == DI - 1))
                nc.scalar.activation(ot_[:, c0 : c0 + cw], p[:IN, :cw], AF.Identity, bias=bo_t[:])
            nc.sync.dma_start(out_d[b], ot_[:])

        for _pool in (pk, sm, wg, sqp, f6, vs, mxp, hqp, fa, w6, res):
            _pool.release()

    nc.compile()
    return nc


def _prep_host(inputs):
    """Build the 8 per-core input maps from full inputs."""
    f32 = np.float32

    def b16(a):
        return np.ascontiguousarray(np.asarray(a, f32)).astype(bf16np)

    def tiled(vec, n):          # [n*128] -> [128, n] (col j = tile j)
        return np.ascontiguousarray(np.asarray(vec, f32).reshape(n, 128).T)

    x = np.asarray(inputs["x"], f32)
    memory = np.asarray(inputs["memory"], f32)
    ts = np.asarray(inputs["timesteps"])
    pe = np.asarray(inputs["pe"], f32)

    half = D // 2
    expo = np.exp(-math.log(10000.0) * np.arange(half, dtype=f32) / (half - 1.0))
    efm = np.concatenate([expo, expo]) / (2 * np.pi)
    phs = np.concatenate([np.zeros(half, f32), np.full(half, 0.25, f32)])

    # alibi steps, kt-tiled: steps_d[kt, k-k0, q]; bias[h] = -slope_h * steps
    di = np.arange(T)[:, None] - np.arange(T)[None, :]
    steps = np.where(di >= 0, di // PERIOD, (-di - 1) // PERIOD).astype(f32)  # [q, j]
    stepsT = np.zeros((TK, T), f32)
    stepsT[1:, :] = steps.T                     # [1+j, q]; row 0 (adapter) = 0
    steps_d = np.zeros((5, 128, T), f32)
    for kt_i, (k0, kwd) in enumerate(KT):
        steps_d[kt_i, :kwd] = stepsT[k0 : k0 + kwd]

    qkv_bias = {}
    for nm in ("sa", "ca"):
        bq = np.asarray(inputs[f"{nm}_bqkv"], f32).copy()      # [L, 3D]
        bq[:, :D] *= INV                                       # pre-scale q bias
        qkv_bias[nm] = np.stack([np.stack([tiled(bq[l, k * 128 : (k + 1) * 128], 1)[:, 0]
                                           for k in range(3 * DI)], axis=1)
                                 for l in range(L)])           # [L,128,24]

    def qkv_c(w):  # [L, 3D, D] -> [L, 3, DI, 128, D] tile-contiguous
        wT = np.asarray(w, f32).transpose(0, 2, 1)             # [L, D, 3D]
        return b16(wT.reshape(L, DI, 128, 3, D).transpose(0, 3, 1, 2, 4))

    common = {
        "tsf": None, "xT": None, "memT_c": None,
        "efm": tiled(efm, DI), "phs": tiled(phs, DI),
        "peT_c": np.ascontiguousarray(
            (pe.T + np.asarray(inputs["b_in"], f32)[:, None]).reshape(DI, 128, T)),
        "w_inT": b16(np.asarray(inputs["W_in"], f32).T),
        "te_w1T_c": b16(np.asarray(inputs["te_W1"], f32).T.reshape(DI, 128, D)),
        "te_w2T_c": b16(np.asarray(inputs["te_W2"], f32).T.reshape(DI, 128, D)),
        "te_b1t": tiled(inputs["te_b1"], DI),
        "te_b2t": tiled(inputs["te_b2"], DI),
        "sa_wqkvT_c": qkv_c(inputs["sa_Wqkv"]),
        "sa_bqkvt": qkv_bias["sa"],
        "sa_bvrow": b16(np.asarray(inputs["sa_bqkv"], f32)[:, 2 * D :][:, None, :]),
        "sa_woT_c": b16(np.asarray(inputs["sa_Wo"], f32).transpose(0, 2, 1).reshape(L, DI, 128, D)),
        "sa_bot": np.stack([tiled(np.asarray(inputs["sa_bo"], f32)[l], DI) for l in range(L)]),
        "ca_wqkvT_c": qkv_c(inputs["ca_Wqkv"]),
        "ca_bqkvt": qkv_bias["ca"],
        "ca_woT_c": b16(np.asarray(inputs["ca_Wo"], f32).transpose(0, 2, 1).reshape(L, DI, 128, D)),
        "ca_bot": np.stack([tiled(np.asarray(inputs["ca_bo"], f32)[l], DI) for l in range(L)]),
        "ff_w1T_c": b16(np.asarray(inputs["ff_W1"], f32).transpose(0, 2, 1)
                        .reshape(L, DI, 128, 4, D).transpose(0, 3, 1, 2, 4)),
        "ff_b1t": np.stack([tiled(np.asarray(inputs["ff_b1"], f32)[l], DFI) for l in range(L)]),
        "ff_w2T_c": b16(np.asarray(inputs["ff_W2"], f32).transpose(0, 2, 1)
                        .reshape(L, 2, 16, 128, D)),
        "ff_b2t": np.stack([tiled(np.asarray(inputs["ff_b2"], f32)[l], DI) for l in range(L)]),
        "lngt": np.stack([np.stack([tiled(np.asarray(inputs[f"ln{k+1}_g"], f32)[l], DI)
                                    for k in range(3)]) for l in range(L)]),
        "lnbt": np.stack([np.stack([tiled(np.asarray(inputs[f"ln{k+1}_b"], f32)[l], DI)
                                    for k in range(3)]) for l in range(L)]),
        "steps_d": steps_d.astype(bf16np),
        "w_outT_c": b16(np.asarray(inputs["W_out"], f32).T.reshape(DI, 128, IN)),
        "b_out": np.asarray(inputs["b_out"], f32)[:, None],
    }

    in_maps = []
    for c in range(NC):
        b0 = c * BC
        m = dict(common)
        m["xT"] = b16(x[b0 : b0 + BC].transpose(0, 2, 1))
        m["memT_c"] = b16(memory[b0 : b0 + BC].transpose(0, 2, 1).reshape(BC, DI, 128, T))
        m["tsf"] = np.asarray(ts[b0 : b0 + BC], f32)[None, :]
        in_maps.append(m)
    return in_maps


def kernel(**inputs):
    from concourse.bass_utils import run_bass_kernel_spmd

    if "nc" not in _cache:
        _cache["nc"] = _build()
    nc = _cache["nc"]
    in_maps = _prep_host(inputs)
    res = run_bass_kernel_spmd(nc, in_maps, core_ids=list(range(NC)))
    out = np.empty((B, T, IN), np.float32)
    for c in range(NC):
        out[c * BC : (c + 1) * BC] = res.results[c]["out"].transpose(0, 2, 1)
    return out


# revision 8
# speedup vs baseline: 1.6377x; 1.0555x over previous
import math
import sys

sys.path.insert(0, "/opt/trn_rl_repo")

import numpy as np
import ml_dtypes

bf16np = ml_dtypes.bfloat16

# ---------------- problem constants (hardcoded; kernel.py must be self-contained) ----
B, T, S, D, H, L, DFF, IN, PERIOD = 16, 600, 600, 1024, 8, 8, 4096, 52, 25
HD = D // H          # 128
NC = 8               # cores
BC = B // NC         # 2 batches per core
DI = D // 128        # 8 i-tiles
DFI = DFF // 128     # 32
INV = 1.0 / math.sqrt(HD)
TK = 1 + T           # 601 keys (adapter + T)
# chunks of the token dim (>=256 wide keeps LDWEIGHTS hidden; <=512 fits a psum bank)
TCH = [(0, 300), (300, 300)]
KCH = [(0, 301), (301, 300)]         # 601-wide
KT = [(0, 128), (128, 128), (256, 128), (384, 128), (512, 89)]  # key tiles of 601
SLOPES = [0.5 ** (h + 1) for h in range(H)]

_cache = {}


def _build():
    """Build the per-core Bass graph (SPMD; same program all 8 cores)."""
    from concourse import bacc, mybir
    import concourse.bass as bass
    import concourse.tile as tile

    f32 = mybir.dt.float32
    bf = mybir.dt.bfloat16
    i32 = mybir.dt.int32
    AF = mybir.ActivationFunctionType
    OP = mybir.AluOpType

    nc = bacc.Bacc("TRN2", target_bir_lowering=False, debug=False, num_devices=NC)

    def din(name, shape, dt=f32):
        return nc.dram_tensor(name, shape, dt, kind="ExternalInput").ap()

    # ---- DRAM inputs (host-prepped layouts; *_c = tile-contiguous) ----
    xT = din("xT", [BC, IN, T], bf)              # x transposed, bf16
    memT_c = din("memT_c", [BC, DI, 128, T], bf)
    tsf = din("tsf", [1, BC])                    # timesteps as f32
    efm = din("efm", [128, DI])                  # e/(2pi) tiled per i-tile col
    phs = din("phs", [128, DI])                  # phase (0 / .25)
    peT_c = din("peT_c", [DI, 128, T])           # pe.T + b_in  (f32)
    w_inT = din("w_inT", [IN, D], bf)
    te_w1T_c = din("te_w1T_c", [DI, 128, D], bf)
    te_w2T_c = din("te_w2T_c", [DI, 128, D], bf)
    te_b1t = din("te_b1t", [128, DI])
    te_b2t = din("te_b2t", [128, DI])
    sa_wqkvT_c = din("sa_wqkvT_c", [L, 3, DI, 128, D], bf)
    sa_bqkvt = din("sa_bqkvt", [L, 128, 3 * DI])  # pre-tiled [128, 24]; q-part prescaled by INV
    sa_bvrow = din("sa_bvrow", [L, 1, D], bf)     # v-bias as row (for ones-MM trick)
    sa_woT_c = din("sa_woT_c", [L, DI, 128, D], bf)
    sa_bot = din("sa_bot", [L, 128, DI])
    ca_wqkvT_c = din("ca_wqkvT_c", [L, 3, DI, 128, D], bf)
    ca_bqkvt = din("ca_bqkvt", [L, 128, 3 * DI])
    ca_woT_c = din("ca_woT_c", [L, DI, 128, D], bf)
    ca_bot = din("ca_bot", [L, 128, DI])
    ff_w1T_c = din("ff_w1T_c", [L, 4, DI, 128, D], bf)
    ff_b1t = din("ff_b1t", [L, 128, DFI])
    ff_w2T_c = din("ff_w2T_c", [L, 2, 16, 128, D], bf)
    ff_b2t = din("ff_b2t", [L, 128, DI])
    lngt = din("lngt", [L, 3, 128, DI])
    lnbt = din("lnbt", [L, 3, 128, DI])
    steps_d = din("steps_d", [5, 128, T], bf)    # alibi steps, kt-tiled [k, q]
    w_outT_c = din("w_outT_c", [DI, 128, IN], bf)
    b_out = din("b_out", [IN, 1])
    out_d = nc.dram_tensor("out", [BC, IN, T], f32, kind="ExternalOutput").ap()

    with tile.TileContext(nc) as tc:
        res = tc.alloc_tile_pool(name="res", bufs=1)      # persistent
        w6 = tc.alloc_tile_pool(name="w6", bufs=26)       # bf16 [128,601] q/k/pt/attn ws
        fa = tc.alloc_tile_pool(name="fa", bufs=17)       # bf16 [128,601] ffa tiles
        hqp = tc.alloc_tile_pool(name="hqp", bufs=10)     # bf16 [128,601] LN targets
        vs = tc.alloc_tile_pool(name="vs", bufs=6)        # bf16 [128,1024] V tiles
        f6 = tc.alloc_tile_pool(name="f6", bufs=7)        # f32 [128,600] workspace
        sqp = tc.alloc_tile_pool(name="sqp", bufs=8)      # bf16 [128,600] LN squares
        wg = tc.alloc_tile_pool(name="wg", bufs=17)       # bf16 [128,1024] weights
        sm = tc.alloc_tile_pool(name="sm", bufs=1)        # small persistents
        pk = tc.alloc_tile_pool(name="pk", bufs=8, space="PSUM")

        # persistent bf16 residual stream (adapter col 0) - ONE batch at a time
        hb1 = [res.tile([128, TK], bf, tag=f"hb_{i}", name=f"hb_{i}") for i in range(DI)]
        hb = [hb1 for _ in range(BC)]
        mx1 = [res.tile([128, TK], bf, tag=f"mx_{i}", name=f"mx_{i}") for i in range(DI)]

        ones_b = sm.tile([1, 128], bf, tag="ones_b", name="ones_b")
        nc.vector.memset(ones_b[:], 1.0)
        ones_b128 = sm.tile([128, 128], bf, tag="ones_b128", name="ones_b128")
        nc.vector.memset(ones_b128[:], 1.0)
        ones_f = sm.tile([1, 128], f32, tag="ones_f", name="ones_f")
        nc.vector.memset(ones_f[:], 1.0)
        eft = sm.tile([128, DI], f32, tag="eft", name="eft")
        nc.sync.dma_start(eft[:], efm[:])
        pht = sm.tile([128, DI], f32, tag="pht", name="pht")
        nc.sync.dma_start(pht[:], phs[:])
        tst = sm.tile([1, BC], f32, tag="tst", name="tst")
        eps_t = sm.tile([128, 1], f32, tag="eps_t", name="eps_t")
        nc.vector.memset(eps_t[:], 1e-5)
        nc.sync.dma_start(tst[:], tsf[:])
        adp = [sm.tile([128, BC], bf, tag=f"adp{i}", name=f"adp{i}") for i in range(DI)]  # adapter bf16
        steps_t = [sm.tile([128, T], bf, tag=f"steps{k}", name=f"steps{k}") for k in range(5)]
        for k in range(5):
            nc.sync.dma_start(steps_t[k][:], steps_d[k])

        # ---------- timestep embedding ----------
        ptb = pk.tile([128, BC], f32, tag="pk", name="ptb")
        nc.tensor.matmul(ptb[:], ones_f[:], tst[:], start=True, stop=True)  # t bcast f32
        temb = []
        for i in range(DI):
            y = sm.tile([128, BC], f32, tag=f"y{i}", name=f"y{i}")
            nc.vector.tensor_scalar_mul(y[:], ptb[:], eft[:, i : i + 1])
            nc.vector.tensor_scalar_add(y[:], y[:], pht[:, i : i + 1])
            yi = sm.tile([128, BC], i32, tag=f"yi{i}", name=f"yi{i}")
            nc.vector.tensor_copy(yi[:], y[:])
            yr = sm.tile([128, BC], f32, tag=f"yr{i}", name=f"yr{i}")
            nc.vector.tensor_copy(yr[:], yi[:])
            fr = sm.tile([128, BC], f32, tag=f"fr{i}", name=f"fr{i}")
            nc.vector.tensor_sub(fr[:], y[:], yr[:])
            tb = sm.tile([128, BC], bf, tag=f"tb{i}", name=f"tb{i}")
            nc.scalar.activation(tb[:], fr[:], AF.Sin, scale=2 * math.pi)
            temb.append(tb)

        def mlp1024(wT_c, bt_d, ins, act, outs_tag):
            """[D,D] proj on BC-wide f-major input tiles. Returns 8 bf16 [128,BC] tiles."""
            bt = sm.tile([128, DI], f32, tag=outs_tag + "_b", name=outs_tag + "_b")
            nc.sync.dma_start(bt[:], bt_d[:])
            ws = []
            for i in range(DI):
                w = wg.tile([128, D], bf, tag="wg", name="wg")
                nc.sync.dma_start(w[:], wT_c[i])
                ws.append(w)
            outs = []
            for o in range(DI):
                p = pk.tile([128, BC], f32, tag="pk", name="pmlp")
                for i in range(DI):
                    nc.tensor.matmul(p[:], ws[i][:, o * 128 : (o + 1) * 128], ins[i][:],
                                     start=(i == 0), stop=(i == DI - 1))
                ob = sm.tile([128, BC], bf, tag=f"{outs_tag}{o}", name=f"{outs_tag}{o}")
                nc.scalar.activation(ob[:], p[:], act, bias=bt[:, o : o + 1])
                outs.append(ob)
            return outs

        z1 = mlp1024(te_w1T_c, te_b1t, temb, AF.Silu, "z1")
        z2 = mlp1024(te_w2T_c, te_b2t, z1, AF.Identity, "z2")
        for i in range(DI):
            nc.vector.tensor_copy(adp[i][:], z2[i][:])

        # ---------- helpers ----------
        def layernorm(b, g_ap, b_ap, tgt=None):
            """LN over features of X held in hb[b][:, 1:] (bf16). Sums via ones-matmul
            reduction. If tgt given, apply writes tgt and hb is back-filled off the
            critical path; else apply is in place on hb."""
            sq = []
            for o in range(DI):
                s = sqp.tile([128, T], bf, tag="sqp", name="sqp")
                nc.vector.tensor_mul(s[:], hb[b][o][:, 1:], hb[b][o][:, 1:])
                sq.append(s)
            m = f6.tile([128, T], f32, tag="f6", name="f6")
            rstd = f6.tile([128, T], f32, tag="f6", name="f6")
            mr = f6.tile([128, T], f32, tag="f6", name="f6")
            for c0, cw in TCH:
                pS = pk.tile([128, 512], f32, tag="pk", name="pk")
                for o in range(DI):
                    nc.tensor.matmul(pS[:, :cw], ones_b128[:],
                                     hb[b][o][:, 1 + c0 : 1 + c0 + cw],
                                     start=(o == 0), stop=(o == DI - 1))
                pS2 = pk.tile([128, 512], f32, tag="pk", name="pk")
                for o in range(DI):
                    nc.tensor.matmul(pS2[:, :cw], ones_b128[:], sq[o][:, c0 : c0 + cw],
                                     start=(o == 0), stop=(o == DI - 1))
                nc.vector.tensor_scalar_mul(m[:, c0 : c0 + cw], pS[:, :cw], 1.0 / D)
                m2 = f6.tile([128, T], f32, tag="f6", name="f6")
                nc.vector.tensor_tensor(m2[:, :cw], m[:, c0 : c0 + cw], m[:, c0 : c0 + cw], OP.mult)
                var = f6.tile([128, T], f32, tag="f6", name="f6")
                nc.vector.scalar_tensor_tensor(var[:, :cw], pS2[:, :cw], 1.0 / D, m2[:, :cw],
                                               OP.mult, OP.subtract)
                sd = f6.tile([128, T], f32, tag="f6", name="f6")
                nc.scalar.activation(sd[:, :cw], var[:, :cw], AF.Sqrt, bias=eps_t[:])
                nc.vector.reciprocal(rstd[:, c0 : c0 + cw], sd[:, :cw])
                nc.vector.tensor_tensor(mr[:, c0 : c0 + cw], m[:, c0 : c0 + cw],
                                        rstd[:, c0 : c0 + cw], OP.mult)
            for o in range(DI):
                dst = tgt[o] if tgt is not None else None
                for c0, cw in TCH:
                    t1 = f6.tile([128, T], f32, tag="f6", name="f6")
                    nc.vector.tensor_tensor(t1[:, :cw], hb[b][o][:, 1 + c0 : 1 + c0 + cw],
                                            rstd[:, c0 : c0 + cw], OP.mult)
                    nc.vector.tensor_tensor(t1[:, :cw], t1[:, :cw], mr[:, c0 : c0 + cw],
                                            OP.subtract)
                    if dst is not None:
                        nc.scalar.activation(dst[:, c0 : c0 + cw], t1[:, :cw], AF.Identity,
                                             bias=b_ap[:, o : o + 1], scale=g_ap[:, o : o + 1])
                    else:
                        nc.scalar.activation(hb[b][o][:, 1 + c0 : 1 + c0 + cw], t1[:, :cw],
                                             AF.Identity,
                                             bias=b_ap[:, o : o + 1], scale=g_ap[:, o : o + 1])
                if dst is not None:
                    nc.vector.tensor_copy(hb[b][o][:, 1:], dst[:, :T])

        def proj_res(b, wT_c_l, bot_ap, rhs_tiles):
            """out-proj [D,D] + bias + residual into hf[b] (X pre-LN)."""
            ws = []
            for i in range(DI):
                w = wg.tile([128, D], bf, tag="wg", name="wg")
                nc.sync.dma_start(w[:], wT_c_l[i])
                ws.append(w)
            for o in range(DI):
                for c0, cw in TCH:
                    p = pk.tile([128, 512], f32, tag="pk", name="pk")
                    for i in range(DI):
                        nc.tensor.matmul(p[:, :cw], ws[i][:, o * 128 : (o + 1) * 128],
                                         rhs_tiles[i][:, c0 : c0 + cw],
                                         start=(i == 0), stop=(i == DI - 1))
                    nc.vector.scalar_tensor_tensor(hb[b][o][:, 1 + c0 : 1 + c0 + cw], p[:, :cw],
                                                   bot_ap[:, o : o + 1],
                                                   hb[b][o][:, 1 + c0 : 1 + c0 + cw],
                                                   OP.add, OP.add)

        # per-layer bias tiles (re-DMAed each (b, l))
        sa_bq = sm.tile([128, 3 * DI], f32, tag="sa_bq", name="sa_bq")
        ca_bq = sm.tile([128, 3 * DI], f32, tag="ca_bq", name="ca_bq")
        sa_bo_t = sm.tile([128, DI], f32, tag="sa_bo_t", name="sa_bo_t")
        ca_bo_t = sm.tile([128, DI], f32, tag="ca_bo_t", name="ca_bo_t")
        f_b1 = sm.tile([128, DFI], f32, tag="f_b1", name="f_b1")
        f_b2 = sm.tile([128, DI], f32, tag="f_b2", name="f_b2")
        lng = [sm.tile([128, DI], f32, tag=f"lng{k}", name=f"lng{k}") for k in range(3)]
        lnb = [sm.tile([128, DI], f32, tag=f"lnb{k}", name=f"lnb{k}") for k in range(3)]
        bvr = sm.tile([1, D], bf, tag="bvr", name="bvr")
        bo_t = sm.tile([IN, 1], f32, tag="bo_t", name="bo_t")
        nc.sync.dma_start(bo_t[:], b_out[:])

        # ================= batch-serial main =================
        for b in range(BC):
            # ---------- input projection + pe ----------
            xb = sm.tile([IN, T], bf, tag="xb", name="xb")
            nc.sync.dma_start(xb[:], xT[b])
            w_in_t = sm.tile([IN, D], bf, tag="w_in_t", name="w_in_t")
            nc.sync.dma_start(w_in_t[:], w_inT[:])
            for o in range(DI):
                pe_t = f6.tile([128, T], f32, tag="f6", name="f6")
                nc.sync.dma_start(pe_t[:], peT_c[o])
                for c0, cw in TCH:
                    p = pk.tile([128, 512], f32, tag="pk", name="pk")
                    nc.tensor.matmul(p[:, :cw], w_in_t[:, o * 128 : (o + 1) * 128],
                                     xb[:, c0 : c0 + cw], start=True, stop=True)
                    nc.vector.tensor_tensor(hb[b][o][:, 1 + c0 : 1 + c0 + cw], p[:, :cw],
                                            pe_t[:, c0 : c0 + cw], OP.add)
                nc.vector.tensor_copy(hb[b][o][:, 0:1], adp[o][:, b : b + 1])
                nc.sync.dma_start(mx1[o][:, 1:], memT_c[b, o])
                nc.vector.tensor_copy(mx1[o][:, 0:1], adp[o][:, b : b + 1])

            for l in range(L):
                nc.sync.dma_start(sa_bq[:], sa_bqkvt[l])
                nc.sync.dma_start(ca_bq[:], ca_bqkvt[l])
                nc.sync.dma_start(sa_bo_t[:], sa_bot[l])
                nc.sync.dma_start(ca_bo_t[:], ca_bot[l])
                nc.sync.dma_start(f_b1[:], ff_b1t[l])
                nc.sync.dma_start(f_b2[:], ff_b2t[l])
                for k in range(3):
                    nc.sync.dma_start(lng[k][:], lngt[l, k])
                    nc.sync.dma_start(lnb[k][:], lnbt[l, k])
                nc.sync.dma_start(bvr[:], sa_bvrow[l])

                # ================= self-attention =================
                def sa_proj(mat, src_off, chunks, bias_off, scale):
                    ws = []
                    for i in range(DI):
                        w = wg.tile([128, D], bf, tag="wg", name="wg")
                        nc.sync.dma_start(w[:], sa_wqkvT_c[l, mat, i])
                        ws.append(w)
                    outs = []
                    for ot in range(DI):
                        dst = w6.tile([128, TK], bf, tag="w6", name="w6")
                        for c0, cw in chunks:
                            p = pk.tile([128, 512], f32, tag="pk", name="pk")
                            for i in range(DI):
                                nc.tensor.matmul(p[:, :cw], ws[i][:, ot * 128 : (ot + 1) * 128],
                                                 hb[b][i][:, src_off + c0 : src_off + c0 + cw],
                                                 start=(i == 0), stop=(i == DI - 1))
                            nc.scalar.activation(dst[:, c0 : c0 + cw], p[:, :cw], AF.Identity,
                                                 bias=sa_bq[:, bias_off + ot : bias_off + ot + 1],
                                                 scale=scale)
                        outs.append(dst)
                    return outs

                qsb = sa_proj(0, 1, TCH, 0, INV)
                ksb = sa_proj(1, 0, KCH, DI, 1.0)
                # V transposed ([key, head*hd]) via hx-stationary matmuls
                vw = []
                for i in range(DI):
                    w = wg.tile([128, D], bf, tag="wg", name="wg")
                    nc.sync.dma_start(w[:], sa_wqkvT_c[l, 2, i])
                    vw.append(w)
                vsb = [vs.tile([128, 1024], bf, tag="vs", name="vs") for _ in range(5)]
                for vc in range(2):
                    vc0 = vc * 512
                    for kt_i, (k0, kwd) in enumerate(KT):
                        p = pk.tile([128, 512], f32, tag="pk", name="pk")
                        for i in range(DI):
                            nc.tensor.matmul(p[:kwd, :], hb[b][i][:, k0 : k0 + kwd],
                                             vw[i][:, vc0 : vc0 + 512],
                                             start=(i == 0), stop=False)
                        nc.tensor.matmul(p[:kwd, :], ones_b[:, :kwd],
                                         bvr[:, vc0 : vc0 + 512], start=False, stop=True)
                        nc.scalar.copy(vsb[kt_i][:kwd, vc0 : vc0 + 512], p[:kwd, :])

                attn = []
                for h in range(H):
                    pts = []
                    for kt_i, (k0, kwd) in enumerate(KT):
                        sx = f6.tile([128, T], f32, tag="f6", name="f6")
                        for c0, cw in TCH:
                            p = pk.tile([128, 512], f32, tag="pk", name="pk")
                            nc.tensor.matmul(p[:kwd, :cw], ksb[h][:, k0 : k0 + kwd],
                                             qsb[h][:, c0 : c0 + cw], start=True, stop=True)
                            nc.vector.scalar_tensor_tensor(sx[:kwd, c0 : c0 + cw],
                                                           steps_t[kt_i][:kwd, c0 : c0 + cw],
                                                           -SLOPES[h], p[:kwd, :cw],
                                                           OP.mult, OP.add)
                        pt = w6.tile([128, TK], bf, tag="w6", name="w6")
                        nc.scalar.activation(pt[:kwd, :T], sx[:kwd, :], AF.Exp)
                        pts.append(pt)
                    rb = f6.tile([128, T], f32, tag="f6", name="f6")
                    for c0, cw in TCH:
                        pd = pk.tile([128, 512], f32, tag="pk", name="pk")
                        for kt_i, (k0, kwd) in enumerate(KT):
                            nc.tensor.matmul(pd[:, :cw], ones_b128[:kwd, :],
                                             pts[kt_i][:kwd, c0 : c0 + cw],
                                             start=(kt_i == 0), stop=(kt_i == 4))
                        nc.vector.reciprocal(rb[:, c0 : c0 + cw], pd[:, :cw])
                    at = w6.tile([128, TK], bf, tag="w6", name="w6")
                    for c0, cw in TCH:
                        p = pk.tile([128, 512], f32, tag="pk", name="pk")
                        for kt_i, (k0, kwd) in enumerate(KT):
                            nc.tensor.matmul(p[:, :cw], vsb[kt_i][:kwd, h * 128 : (h + 1) * 128],
                                             pts[kt_i][:kwd, c0 : c0 + cw],
                                             start=(kt_i == 0), stop=(kt_i == 4))
                        nc.vector.tensor_tensor(at[:, c0 : c0 + cw], p[:, :cw],
                                                rb[:, c0 : c0 + cw], OP.mult)
                    attn.append(at)
                proj_res(b, sa_woT_c[l], sa_bo_t[:], attn)
                layernorm(b, lng[0][:], lnb[0][:])

                # ================= cross-attention =================
                def ca_proj(mat, src_tiles, src_off, chunks, bias_off, scale):
                    ws = []
                    for i in range(DI):
                        w = wg.tile([128, D], bf, tag="wg", name="wg")
                        nc.sync.dma_start(w[:], ca_wqkvT_c[l, mat, i])
                        ws.append(w)
                    outs = []
                    for ot in range(DI):
                        dst = w6.tile([128, TK], bf, tag="w6", name="w6")
                        for c0, cw in chunks:
                            p = pk.tile([128, 512], f32, tag="pk", name="pk")
                            for i in range(DI):
                                nc.tensor.matmul(p[:, :cw], ws[i][:, ot * 128 : (ot + 1) * 128],
                                                 src_tiles[i][:, src_off + c0 : src_off + c0 + cw],
                                                 start=(i == 0), stop=(i == DI - 1))
                            nc.scalar.activation(dst[:, c0 : c0 + cw], p[:, :cw], AF.Identity,
                                                 bias=ca_bq[:, bias_off + ot : bias_off + ot + 1],
                                                 scale=scale)
                        outs.append(dst)
                    return outs

                # k/v first: they depend only on memory, so they overlap LN1
                ks_ca = ca_proj(1, mx1, 0, KCH, DI, 1.0)
                vs_ca = ca_proj(2, mx1, 0, KCH, 2 * DI, 1.0)
                qs_ca = ca_proj(0, hb[b], 1, TCH, 0, INV)

                ca_attn = []
                for h in range(H):
                    kh, vh, qh = ks_ca[h], vs_ca[h], qs_ca[h]
                    ka = f6.tile([128, 1], f32, tag="ka", name="ka")
                    nc.vector.tensor_copy(ka[:], kh[:, 0:1])
                    va = f6.tile([128, 1], f32, tag="ka", name="va")
                    nc.vector.tensor_copy(va[:], vh[:, 0:1])
                    kd = w6.tile([128, TK], bf, tag="w6", name="w6")
                    nc.vector.tensor_scalar_sub(kd[:, :T], kh[:, 1:], ka[:])
                    e = w6.tile([128, TK], bf, tag="w6", name="w6")
                    nc.vector.tensor_tensor(e[:, :T], qh[:, :T], kd[:, :T], OP.mult)
                    wm = w6.tile([128, TK], bf, tag="w6", name="w6")
                    for c0, cw in TCH:
                        pd = pk.tile([128, 512], f32, tag="pk", name="pk")
                        nc.tensor.matmul(pd[:, :cw], ones_b128[:], e[:, c0 : c0 + cw],
                                         start=True, stop=True)
                        nc.scalar.activation(wm[:, c0 : c0 + cw], pd[:, :cw], AF.Sigmoid)
                    vd = w6.tile([128, TK], bf, tag="w6", name="w6")
                    nc.vector.tensor_scalar_sub(vd[:, :T], vh[:, 1:], va[:])
                    at = w6.tile([128, TK], bf, tag="w6", name="w6")
                    nc.vector.tensor_tensor(at[:, :T], vd[:, :T], wm[:, :T], OP.mult)
                    nc.vector.tensor_scalar_add(at[:, :T], at[:, :T], va[:])
                    ca_attn.append(at)
                proj_res(b, ca_woT_c[l], ca_bo_t[:], ca_attn)
                hq = [hqp.tile([128, TK], bf, tag="hqp", name="hqp") for _ in range(DI)]
                layernorm(b, lng[1][:], lnb[1][:], hq)

                # ================= FFN =================
                for half in range(2):
                    ffa = []
                    for g2 in range(2):
                        gi = half * 2 + g2
                        w1s = []
                        for i in range(DI):
                            w = wg.tile([128, D], bf, tag="wg", name="wg")
                            nc.sync.dma_start(w[:], ff_w1T_c[l, gi, i])
                            w1s.append(w)
                        for ot in range(DI):
                            o = gi * DI + ot
                            dst = fa.tile([128, TK], bf, tag="fa", name="fa")
                            for c0, cw in TCH:
                                p = pk.tile([128, 512], f32, tag="pk", name="pk")
                                for i in range(DI):
                                    nc.tensor.matmul(p[:, :cw], w1s[i][:, ot * 128 : (ot + 1) * 128],
                                                     hq[i][:, c0 : c0 + cw],
                                                     start=(i == 0), stop=(i == DI - 1))
                                nc.scalar.activation(dst[:, c0 : c0 + cw], p[:, :cw], AF.Relu,
                                                     bias=f_b1[:, o : o + 1])
                            ffa.append(dst)
                    w2s = []
                    for ii in range(16):
                        w = wg.tile([128, D], bf, tag="wg", name="wg")
                        nc.sync.dma_start(w[:], ff_w2T_c[l, half, ii])
                        w2s.append(w)
                    for o in range(DI):
                        for c0, cw in TCH:
                            p = pk.tile([128, 512], f32, tag="pk", name="pk")
                            for ii in range(16):
                                nc.tensor.matmul(p[:, :cw], w2s[ii][:, o * 128 : (o + 1) * 128],
                                                 ffa[ii][:, c0 : c0 + cw],
                                                 start=(ii == 0), stop=(ii == 15))
                            if half == 0:
                                nc.vector.scalar_tensor_tensor(
                                    hb[b][o][:, 1 + c0 : 1 + c0 + cw], p[:, :cw],
                                    f_b2[:, o : o + 1], hb[b][o][:, 1 + c0 : 1 + c0 + cw],
                                    OP.add, OP.add)
                            else:
                                nc.vector.tensor_tensor(hb[b][o][:, 1 + c0 : 1 + c0 + cw],
                                                        p[:, :cw],
                                                        hb[b][o][:, 1 + c0 : 1 + c0 + cw],
                                                        OP.add)
                layernorm(b, lng[2][:], lnb[2][:])

            # ---------- output projection ----------
            wo_t = []
            for i in range(DI):
                w = wg.tile([128, IN], bf, tag="wgout", name="wgout")
                nc.sync.dma_start(w[:], w_outT_c[i])
                wo_t.append(w)
            ot_ = sm.tile([IN, T], f32, tag=f"osb{b}", name=f"osb{b}")
            for c0, cw in TCH:
                p = pk.tile([128, 512], f32, tag="pk", name="pk")
                for i in range(DI):
                    nc.tensor.matmul(p[:IN, :cw], wo_t[i][:], hb[b][i][:, 1 + c0 : 1 + c0 + cw],
                                     start=(i == 0), stop=(i == DI - 1))
                nc.scalar.activation(ot_[:, c0 : c0 + cw], p[:IN, :cw], AF.Identity, bias=bo_t[:])
            nc.sync.dma_start(out_d[b], ot_[:])

        for _pool in (pk, sm, wg, sqp, f6, vs, hqp, fa, w6, res):
            _pool.release()

    nc.compile()
    return nc


def _prep_host(inputs):
    """Build the 8 per-core input maps from full inputs."""
    f32 = np.float32

    def b16(a):
        return np.ascontiguousarray(np.asarray(a, f32)).astype(bf16np)

    def tiled(vec, n):          # [n*128] -> [128, n] (col j = tile j)
        return np.ascontiguousarray(np.asarray(vec, f32).reshape(n, 128).T)

    x = np.asarray(inputs["x"], f32)
    memory = np.asarray(inputs["memory"], f32)
    ts = np.asarray(inputs["timesteps"])
    pe = np.asarray(inputs["pe"], f32)

    half = D // 2
    expo = np.exp(-math.log(10000.0) * np.arange(half, dtype=f32) / (half - 1.0))
    efm = np.concatenate([expo, expo]) / (2 * np.pi)
    phs = np.concatenate([np.zeros(half, f32), np.full(half, 0.25, f32)])

    # alibi steps, kt-tiled: steps_d[kt, k-k0, q]; bias[h] = -slope_h * steps
    di = np.arange(T)[:, None] - np.arange(T)[None, :]
    steps = np.where(di >= 0, di // PERIOD, (-di - 1) // PERIOD).astype(f32)  # [q, j]
    stepsT = np.zeros((TK, T), f32)
    stepsT[1:, :] = steps.T                     # [1+j, q]; row 0 (adapter) = 0
    steps_d = np.zeros((5, 128, T), f32)
    for kt_i, (k0, kwd) in enumerate(KT):
        steps_d[kt_i, :kwd] = stepsT[k0 : k0 + kwd]

    qkv_bias = {}
    for nm in ("sa", "ca"):
        bq = np.asarray(inputs[f"{nm}_bqkv"], f32).copy()      # [L, 3D]
        bq[:, :D] *= INV                                       # pre-scale q bias
        qkv_bias[nm] = np.stack([np.stack([tiled(bq[l, k * 128 : (k + 1) * 128], 1)[:, 0]
                                           for k in range(3 * DI)], axis=1)
                                 for l in range(L)])           # [L,128,24]

    def qkv_c(w):  # [L, 3D, D] -> [L, 3, DI, 128, D] tile-contiguous
        wT = np.asarray(w, f32).transpose(0, 2, 1)             # [L, D, 3D]
        return b16(wT.reshape(L, DI, 128, 3, D).transpose(0, 3, 1, 2, 4))

    common = {
        "tsf": None, "xT": None, "memT_c": None,
        "efm": tiled(efm, DI), "phs": tiled(phs, DI),
        "peT_c": np.ascontiguousarray(
            (pe.T + np.asarray(inputs["b_in"], f32)[:, None]).reshape(DI, 128, T)),
        "w_inT": b16(np.asarray(inputs["W_in"], f32).T),
        "te_w1T_c": b16(np.asarray(inputs["te_W1"], f32).T.reshape(DI, 128, D)),
        "te_w2T_c": b16(np.asarray(inputs["te_W2"], f32).T.reshape(DI, 128, D)),
        "te_b1t": tiled(inputs["te_b1"], DI),
        "te_b2t": tiled(inputs["te_b2"], DI),
        "sa_wqkvT_c": qkv_c(inputs["sa_Wqkv"]),
        "sa_bqkvt": qkv_bias["sa"],
        "sa_bvrow": b16(np.asarray(inputs["sa_bqkv"], f32)[:, 2 * D :][:, None, :]),
        "sa_woT_c": b16(np.asarray(inputs["sa_Wo"], f32).transpose(0, 2, 1).reshape(L, DI, 128, D)),
        "sa_bot": np.stack([tiled(np.asarray(inputs["sa_bo"], f32)[l], DI) for l in range(L)]),
        "ca_wqkvT_c": qkv_c(inputs["ca_Wqkv"]),
        "ca_bqkvt": qkv_bias["ca"],
        "ca_woT_c": b16(np.asarray(inputs["ca_Wo"], f32).transpose(0, 2, 1).reshape(L, DI, 128, D)),
        "ca_bot": np.stack([tiled(np.asarray(inputs["ca_bo"], f32)[l], DI) for l in range(L)]),
        "ff_w1T_c": b16(np.asarray(inputs["ff_W1"], f32).transpose(0, 2, 1)
                        .reshape(L, DI, 128, 4, D).transpose(0, 3, 1, 2, 4)),
        "ff_b1t": np.stack([tiled(np.asarray(inputs["ff_b1"], f32)[l], DFI) for l in range(L)]),
        "ff_w2T_c": b16(np.asarray(inputs["ff_W2"], f32).transpose(0, 2, 1)
                        .reshape(L, 2, 16, 128, D)),
        "ff_b2t": np.stack([tiled(np.asarray(inputs["ff_b2"], f32)[l], DI) for l in range(L)]),
        "lngt": np.stack([np.stack([tiled(np.asarray(inputs[f"ln{k+1}_g"], f32)[l], DI)
                                    for k in range(3)]) for l in range(L)]),
        "lnbt": np.stack([np.stack([tiled(np.asarray(inputs[f"ln{k+1}_b"], f32)[l], DI)
                                    for k in range(3)]) for l in range(L)]),
        "steps_d": steps_d.astype(bf16np),
        "w_outT_c": b16(np.asarray(inputs["W_out"], f32).T.reshape(DI, 128, IN)),
        "b_out": np.asarray(inputs["b_out"], f32)[:, None],
    }

    in_maps = []
    for c in range(NC):
        b0 = c * BC
        m = dict(common)
        m["xT"] = b16(x[b0 : b0 + BC].transpose(0, 2, 1))
        m["memT_c"] = b16(memory[b0 : b0 + BC].transpose(0, 2, 1).reshape(BC, DI, 128, T))
        m["tsf"] = np.asarray(ts[b0 : b0 + BC], f32)[None, :]
        in_maps.append(m)
    return in_maps


def kernel(**inputs):
    from concourse.bass_utils import run_bass_kernel_spmd

    if "nc" not in _cache:
        _cache["nc"] = _build()
    nc = _cache["nc"]
    in_maps = _prep_host(inputs)
    res = run_bass_kernel_spmd(nc, in_maps, core_ids=list(range(NC)))
    out = np.empty((B, T, IN), np.float32)
    for c in range(NC):
        out[c * BC : (c + 1) * BC] = res.results[c]["out"].transpose(0, 2, 1)
    return out
